# revision 1
# baseline (speedup 1.0000x reference)
"""Block-sparse local+strided attention (LocalStridedBlockSparseAttn) on 8 trn2 cores.

Problem: q,k,v [4096, 16, 64] f32, single prefill sequence. Per-head block mask
(64x64 token blocks): j <= i and (i - j < 8  or  (j + h + 1) % 8 == 0).

Sharding: core c owns heads {c, c+8} — both have the same strided residue
s = (7 - c) % 8, so one SPMD program serves all 8 cores with per-core data:
  - local part  (i-j <= 7, any j): uses natural block layout, fixed masks
  - strided part (j ≡ s mod 8, i-j >= 8): host packs the 7 candidate strided
    k/v blocks contiguously; validity boundary (i >= j+8) lands in q-chunk b+1
    at column offset s*64, masked by a per-core bias row added pre-exp.

Math per head: scoresT[n, m] = K @ q^T (d on partitions for both operands),
exp (no max subtraction; scores ~ N(0,1) so fp32-safe), P^T used directly as
the PV matmul's stationary operand with [V | 1] augmented rhs... actually
lhsT = [V | ones] so out = [V|1]^T @ P^T = [O^T; rowsums] accumulated in PSUM,
then transpose back with PE and normalize by reciprocal(rowsums).
"""

import numpy as np

N_HEADS = 16
HEAD = 64
SEQ = 4096
BS = 64
NB = 64          # 64 token-blocks
LOCAL = 8
VERT = 8
SM_SCALE = 1.0 / 8.0
NCORES = 8
CHUNK = 512      # q tokens per chunk (8 blocks)
NCH = SEQ // CHUNK
NSB = 7          # packed strided blocks per core (b=7 never valid)

_cache = {}


def _legalize_waits(nc, max_waits=1):
    """This walrus build rejects instructions carrying more than one sync-wait
    condition ("Too many sync wait commands"); hoist extras into same-engine
    NoOps placed immediately before the instruction."""
    import concourse.mybir as mybir

    nid = 0
    for bb in nc.main_func.blocks:
        new = []
        for ins in bb.instructions:
            si = ins.sync_info
            if si is not None and si.on_wait and len(si.on_wait) > max_waits:
                waits = list(si.on_wait)
                while len(waits) > max_waits:
                    chunk, waits = waits[:max_waits], waits[max_waits:]
                    nid += 1
                    nop = mybir.InstNoOp(name=f"{ins.name}-wsplit{nid}")
                    nop.engine = ins.engine
                    nop.sync_info = mybir.SyncInfo(on_wait=chunk, on_update=[])
                    new.append(nop)
                ins.sync_info = mybir.SyncInfo(on_wait=waits,
                                               on_update=list(si.on_update))
            new.append(ins)
        bb.instructions[:] = new
    return nc


def _build_program(chunks=None, heads=(0, 1)):
    from contextlib import ExitStack

    import concourse.bass as bass
    import concourse.mybir as mybir
    from concourse import tile

    f32 = mybir.dt.float32
    Exp = mybir.ActivationFunctionType.Exp

    nc = bass.Bass()
    qh = nc.dram_tensor("qh", [SEQ, 128], f32, kind="ExternalInput")
    kh = nc.dram_tensor("kh", [SEQ, 128], f32, kind="ExternalInput")
    vh = nc.dram_tensor("vh", [SEQ, 128], f32, kind="ExternalInput")
    ks = nc.dram_tensor("ks", [NSB * BS, 128], f32, kind="ExternalInput")
    vs = nc.dram_tensor("vs", [NSB * BS, 128], f32, kind="ExternalInput")
    biasrow = nc.dram_tensor("biasrow", [1, CHUNK], f32, kind="ExternalInput")
    outh = nc.dram_tensor("outh", [SEQ, 128], f32, kind="ExternalOutput")

    # Device-constant tiles (same on every core).
    m0_np = np.zeros((128, 64), np.float32)   # k-pair rows vs q-block 2a
    m1_np = np.zeros((128, 64), np.float32)   # k-pair rows vs q-block 2a+1
    n = np.arange(64)
    m0_np[:64] = (n[None, :] >= n[:, None]).astype(np.float32)  # tri; bottom 0
    m1_np[:64] = 1.0
    m1_np[64:] = (n[None, :] >= n[:, None]).astype(np.float32)
    ident_d = nc.inline_tensor(np.eye(128, dtype=np.float32), "ident_c")
    m0_d = nc.inline_tensor(m0_np, "m0_c")
    m1_d = nc.inline_tensor(m1_np, "m1_c")
    ones_d = nc.inline_tensor(np.ones((1, 64), np.float32), "ones_c")

    with tile.TileContext(nc) as tc, ExitStack() as ctx:
        const = ctx.enter_context(tc.tile_pool(name="const", bufs=1))
        ident = const.tile([128, 128], f32, tag="ident")
        m0 = const.tile([128, 64], f32, tag="m0")
        m1 = const.tile([128, 64], f32, tag="m1")
        ones1 = const.tile([1, 64], f32, tag="ones1")
        biasSB = const.tile([1, CHUNK], f32, tag="bias")
        nc.sync.dma_start(ident[:], ident_d[:])
        nc.sync.dma_start(m0[:], m0_d[:])
        nc.sync.dma_start(m1[:], m1_d[:])
        nc.sync.dma_start(ones1[:], ones_d[:])
        nc.sync.dma_start(biasSB[:], biasrow[:])

        big = ctx.enter_context(tc.tile_pool(name="big", bufs=1))
        qT = big.tile([128, SEQ], f32, tag="qT")      # [(h,d), t]
        kT = big.tile([128, SEQ], f32, tag="kT")
        ksT = big.tile([128, NSB * BS], f32, tag="ksT")
        # vaug: per 128-token pair a: [V_h0(64) | 1 | V_h1(64) | 1] = 130 cols
        vaug = big.tile([128, 32 * 130], f32, tag="vaug")
        # vsaug: packed strided block b (64 tokens): same 130-col chunks
        vsaug = big.tile([64, NSB * 130], f32, tag="vsaug")
        # vtail: odd blocks 2a+1 at partition base 0 (tail pieces' lhsT)
        vtail = big.tile([64, 32 * 130], f32, tag="vtail")

        # ---- Stage A: load + PE-transpose q, k (and packed strided k) ----
        with tc.tile_pool(name="ld", bufs=4) as ld, \
             tc.tile_pool(name="psA", bufs=2, space="PSUM") as psA:
            for src, dstT in ((qh, qT), (kh, kT)):
                for t in range(32):
                    tl = ld.tile([128, 128], f32, tag="ld")
                    nc.sync.dma_start(tl[:], src[t * 128:(t + 1) * 128, :])
                    pt = psA.tile([128, 128], f32, tag="psA")
                    nc.tensor.transpose(pt[:], tl[:], ident[:])
                    nc.vector.tensor_copy(dstT[:, t * 128:(t + 1) * 128], pt[:])
            for t in range(4):
                rows = 128 if t < 3 else 64
                tl = ld.tile([128, 128], f32, tag="ld")
                nc.sync.dma_start(tl[:rows, :], ks[t * 128:t * 128 + rows, :])
                pt = psA.tile([128, 128], f32, tag="psA")
                nc.tensor.transpose(pt[:, :rows], tl[:rows, :], ident[:rows, :rows])
                nc.vector.tensor_copy(ksT[:, t * 128:t * 128 + rows], pt[:, :rows])

        # ---- v into augmented layouts + ones columns ----
        for h in range(2):
            nc.sync.dma_start(
                vaug[:].rearrange("p (a c) -> p a c", c=130)[:, :, h * 65:h * 65 + 64],
                vh[:].rearrange("(a p) hd -> p a hd", p=128)[:, :, h * 64:(h + 1) * 64])
            nc.sync.dma_start(
                vsaug[:].rearrange("p (b c) -> p b c", c=130)[:, :, h * 65:h * 65 + 64],
                vs[:].rearrange("(b p) hd -> p b hd", p=64)[:, :, h * 64:(h + 1) * 64])
            nc.sync.dma_start(
                vtail[:].rearrange("p (a c) -> p a c", c=130)[:, :, h * 65:h * 65 + 64],
                vh[:].rearrange("(a two p) hd -> p a two hd", two=2, p=64)
                    [:, :, 1, h * 64:(h + 1) * 64])
        nc.vector.memset(vaug[:].rearrange("p (g c) -> p g c", c=65)[:, :, 64:65], 1.0)
        nc.vector.memset(vsaug[:].rearrange("p (g c) -> p g c", c=65)[:, :, 64:65], 1.0)
        nc.vector.memset(vtail[:].rearrange("p (g c) -> p g c", c=65)[:, :, 64:65], 1.0)

        # ---- Stage B: chunked attention ----
        expp = ctx.enter_context(tc.tile_pool(name="expp", bufs=26))
        psS = ctx.enter_context(tc.tile_pool(name="psS", bufs=4, space="PSUM"))
        psOT = ctx.enter_context(tc.tile_pool(name="psOT", bufs=2, space="PSUM"))
        psT = ctx.enter_context(tc.tile_pool(name="psT", bufs=2, space="PSUM"))
        sot = ctx.enter_context(tc.tile_pool(name="sot", bufs=2))
        outp = ctx.enter_context(tc.tile_pool(name="outp", bufs=8))
        rcp = ctx.enter_context(tc.tile_pool(name="rcp", bufs=4))

        for c in (range(NCH) if chunks is None else chunks):
            otiles = [outp.tile([128, 128], f32, tag="ot", name=f"ot{c}_{k4}")
                      for k4 in range(4)]
            for h in heads:
                hq = slice(h * 64, (h + 1) * 64)
                hv = h * 65
                ot = psOT.tile([65, CHUNK], f32, tag="psOT")
                nc.vector.memset(ot[:], 0.0)
                pieces = []  # (expT ap, n, lhsT ap, out_col, width)

                # strided: packed block b valid for chunks c >= b+1; the
                # boundary chunk (c == b+1) gets the -1e9 bias on cols < s*64.
                for b in range(NSB):
                    if c < b + 1:
                        continue
                    ps = psS.tile([64, CHUNK], f32, tag="psS")
                    bnd = (c == b + 1)
                    nc.tensor.matmul(ps[:], ksT[hq, b * 64:(b + 1) * 64],
                                     qT[hq, c * CHUNK:(c + 1) * CHUNK],
                                     start=True, stop=not bnd)
                    if bnd:
                        nc.tensor.matmul(ps[:], ones1[:], biasSB[:],
                                         start=False, stop=True)
                    et = expp.tile([64, CHUNK], f32, tag="exp")
                    nc.scalar.activation(et[:], ps[:], Exp, scale=SM_SCALE)
                    pieces.append((et[:], vsaug[:, b * 130 + hv:b * 130 + hv + 65],
                                   0, CHUNK))

                # local diagonal column-blocks (masked)
                for a in range(4 * c, 4 * c + 4):
                    for d01 in range(2):
                        i = 2 * a + d01
                        ps = psS.tile([128, 64], f32, tag="psS")
                        nc.tensor.matmul(ps[:], kT[hq, a * 128:(a + 1) * 128],
                                         qT[hq, i * 64:(i + 1) * 64],
                                         start=True, stop=True)
                        et = expp.tile([128, 64], f32, tag="exp")
                        nc.scalar.activation(et[:], ps[:], Exp, scale=SM_SCALE)
                        nc.vector.tensor_mul(et[:], et[:], (m0 if d01 == 0 else m1)[:])
                        pieces.append((et[:], vaug[:, a * 130 + hv:a * 130 + hv + 65],
                                       (i - 8 * c) * 64, 64))

                # local fully-allowed: k-pair a vs q-blocks [2a+2, 2a+7]
                for a in range(max(0, 4 * c - 3), min(31, 4 * c + 2) + 1):
                    lo = max(2 * a + 2, 8 * c)
                    hi = min(2 * a + 7, 8 * c + 7, 63)
                    if lo > hi:
                        continue
                    w = (hi - lo + 1) * 64
                    ps = psS.tile([128, CHUNK], f32, tag="psS")
                    nc.tensor.matmul(ps[:, :w], kT[hq, a * 128:(a + 1) * 128],
                                     qT[hq, lo * 64:(hi + 1) * 64],
                                     start=True, stop=True)
                    et = expp.tile([128, CHUNK], f32, tag="exp")
                    nc.scalar.activation(et[:, :w], ps[:, :w], Exp, scale=SM_SCALE)
                    pieces.append((et[:, :w], vaug[:, a * 130 + hv:a * 130 + hv + 65],
                                   (lo - 8 * c) * 64, w))

                # local tails: (j = 2a+1, i = 2a+8), i-j = 7
                for a in range(max(0, 4 * c - 4), 4 * c):
                    i = 2 * a + 8
                    if not (8 * c <= i <= min(8 * c + 7, 63)):
                        continue
                    ps = psS.tile([64, 64], f32, tag="psS")
                    nc.tensor.matmul(ps[:], kT[hq, (2 * a + 1) * 64:(2 * a + 2) * 64],
                                     qT[hq, i * 64:(i + 1) * 64],
                                     start=True, stop=True)
                    et = expp.tile([64, 64], f32, tag="exp")
                    nc.scalar.activation(et[:], ps[:], Exp, scale=SM_SCALE)
                    pieces.append((et[:], vtail[:, a * 130 + hv:a * 130 + hv + 65],
                                   (i - 8 * c) * 64, 64))

                # PV accumulation onto the memset PSUM tile; pieces touch
                # different column sub-ranges so none may use start=True.
                for pi, (et, vl, col, w) in enumerate(pieces):
                    nc.tensor.matmul(ot[:, col:col + w], vl, et,
                                     start=False, stop=(pi == len(pieces) - 1),
                                     skip_group_check=True)

                so = sot.tile([65, CHUNK], f32, tag="sot")
                nc.vector.tensor_copy(so[:], ot[:])
                for k4 in range(4):
                    pt = psT.tile([128, 65], f32, tag="psT")
                    nc.tensor.transpose(pt[:], so[:, k4 * 128:(k4 + 1) * 128],
                                        ident[:65, :65])
                    rc = rcp.tile([128, 1], f32, tag="rcp")
                    nc.vector.reciprocal(rc[:], pt[:, 64:65])
                    nc.vector.tensor_scalar_mul(
                        otiles[k4][:, h * 64:(h + 1) * 64], pt[:, 0:64], rc[:])

            for k4 in range(4):
                nc.sync.dma_start(
                    outh[c * CHUNK + k4 * 128:c * CHUNK + (k4 + 1) * 128, :],
                    otiles[k4][:])

    return nc


def _in_maps(q, k, v):
    maps = []
    for c in range(NCORES):
        heads = [c, c + 8]
        s = (7 - c) % 8
        qhc = np.ascontiguousarray(q[:, heads, :].reshape(SEQ, 128))
        khc = np.ascontiguousarray(k[:, heads, :].reshape(SEQ, 128))
        vhc = np.ascontiguousarray(v[:, heads, :].reshape(SEQ, 128))
        sj = [s + 8 * b for b in range(NSB)]
        ksc = np.ascontiguousarray(
            np.concatenate([k[j * BS:(j + 1) * BS, heads, :] for j in sj]
            ).reshape(NSB * BS, 128))
        vsc = np.ascontiguousarray(
            np.concatenate([v[j * BS:(j + 1) * BS, heads, :] for j in sj]
            ).reshape(NSB * BS, 128))
        bias = np.zeros((1, CHUNK), np.float32)
        bias[0, :s * 64] = -1e9
        maps.append({"qh": qhc, "kh": khc, "vh": vhc,
                     "ks": ksc, "vs": vsc, "biasrow": bias})
    return maps


def kernel(q, k, v, cu_seqlens_k=None, **_):
    from concourse.bass_utils import run_bass_kernel_spmd

    q = np.asarray(q, np.float32)
    k = np.asarray(k, np.float32)
    v = np.asarray(v, np.float32)
    if "nc" not in _cache:
        _cache["nc"] = _legalize_waits(_build_program())
    res = run_bass_kernel_spmd(_cache["nc"], _in_maps(q, k, v),
                               list(range(NCORES))).results
    out = np.empty((SEQ, N_HEADS, HEAD), np.float32)
    for c in range(NCORES):
        o = res[c]["outh"].reshape(SEQ, 2, HEAD)
        out[:, c, :] = o[:, 0, :]
        out[:, c + 8, :] = o[:, 1, :]
    return out



# revision 5
# speedup vs baseline: 2.0084x; 2.0084x over previous
"""Block-sparse local+strided attention (LocalStridedBlockSparseAttn) on 8 trn2 cores.

Problem: q,k,v [4096, 16, 64] f32, single prefill sequence. Per-head block mask
(64x64 token blocks): j <= i and (i - j < 8  or  (j + h + 1) % 8 == 0).

Sharding: core c owns heads {c, c+8} - both have the same strided residue
s = (7 - c) % 8, so one SPMD program serves all 8 cores with per-core data.

v2 (this file): all matmul operands bf16 (4x PE throughput vs fp32), strided
k-blocks packed in PAIRS onto 128 partitions (halves strided matmul + exp
columns), and all layout prep (transposes, V augmentation with ones columns)
moved to the host so the kernel is pure compute + thin DMA:
  - host ships qT,kT [128, 4096] bf16 (head-dim x 2 heads on partitions),
    ksT [128, 512] (8 packed strided blocks, last zero pad), and V in three
    augmented layouts ([V_h0 | 1 | V_h1 | 1] column groups).
  - scoresT[n, m] = K @ q^T per piece, exp via ACT (scale=1/8, bf16 out),
    P^T streams into PV matmul with [V | 1] stationary -> [O^T; rowsums] in
    PSUM, PE-transpose back, normalize by reciprocal(rowsums).
  - strided validity boundary (i >= j+8) lands in q-chunk b+1 at column
    offset s*64; masked by a -1e9 bias row accumulated pre-exp via a 1-row
    matmul whose lhsT selects the boundary block's partition half.
"""

import numpy as np

N_HEADS = 16
HEAD = 64
SEQ = 4096
BS = 64
NB = 64          # 64 token-blocks
LOCAL = 8
VERT = 8
SM_SCALE = 1.0 / 8.0
NCORES = 8
CHUNK = 512      # q tokens per chunk (8 blocks)
NCH = SEQ // CHUNK
NSB = 8          # packed strided block slots (7 real, slot 7 zero pad)

_cache = {}


def _legalize_waits(nc, max_waits=1):
    """This walrus build rejects instructions carrying more than one sync-wait
    condition ("Too many sync wait commands"); hoist extras into same-engine
    NoOps placed immediately before the instruction."""
    import concourse.mybir as mybir

    nid = 0
    for bb in nc.main_func.blocks:
        new = []
        for ins in bb.instructions:
            si = ins.sync_info
            if si is not None and si.on_wait and len(si.on_wait) > max_waits:
                waits = list(si.on_wait)
                while len(waits) > max_waits:
                    chunk, waits = waits[:max_waits], waits[max_waits:]
                    nid += 1
                    nop = mybir.InstNoOp(name=f"{ins.name}-wsplit{nid}")
                    nop.engine = ins.engine
                    nop.sync_info = mybir.SyncInfo(on_wait=chunk, on_update=[])
                    new.append(nop)
                ins.sync_info = mybir.SyncInfo(on_wait=waits,
                                               on_update=list(si.on_update))
            new.append(ins)
        bb.instructions[:] = new
    return nc


def _build_program(chunks=None, heads=(0, 1)):
    from contextlib import ExitStack

    import concourse.bass as bass
    import concourse.mybir as mybir
    from concourse import tile

    f32 = mybir.dt.float32
    bf16 = mybir.dt.bfloat16
    Exp = mybir.ActivationFunctionType.Exp

    nc = bass.Bass()
    qT_d = nc.dram_tensor("qT", [128, SEQ], bf16, kind="ExternalInput")
    kT_d = nc.dram_tensor("kT", [128, SEQ], bf16, kind="ExternalInput")
    ksT_d = nc.dram_tensor("ksT", [128, NSB * BS], bf16, kind="ExternalInput")
    vaug_d = nc.dram_tensor("vaug", [128, 32 * 130], bf16, kind="ExternalInput")
    vsaug_d = nc.dram_tensor("vsaug", [128, 4 * 130], bf16, kind="ExternalInput")
    vtail_d = nc.dram_tensor("vtail", [64, 32 * 130], bf16, kind="ExternalInput")
    biasrow = nc.dram_tensor("biasrow", [1, CHUNK], bf16, kind="ExternalInput")
    outh = nc.dram_tensor("outh", [SEQ, 128], f32, kind="ExternalOutput")

    # Device-constant tiles (same on every core).
    m0_np = np.zeros((128, 64), np.float32)   # k-pair rows vs q-block 2a
    m1_np = np.zeros((128, 64), np.float32)   # k-pair rows vs q-block 2a+1
    n = np.arange(64)
    m0_np[:64] = (n[None, :] >= n[:, None]).astype(np.float32)  # tri; bottom 0
    m1_np[:64] = 1.0
    m1_np[64:] = (n[None, :] >= n[:, None]).astype(np.float32)
    import ml_dtypes
    bf = ml_dtypes.bfloat16
    ident_d = nc.inline_tensor(np.eye(128, dtype=np.float32), "ident_c")
    m0_d = nc.inline_tensor(m0_np.astype(bf), "m0_c")
    m1_d = nc.inline_tensor(m1_np.astype(bf), "m1_c")
    ones_d = nc.inline_tensor(np.ones((1, 64), bf), "ones_c")
    # selector rows for the boundary bias, side by side in columns:
    # cols 0-127 = single piece (ones on first 64), cols 128-255 = pair
    # with boundary in high half (ones on last 64)
    sel_np = np.zeros((1, 256), bf)
    sel_np[0, :64] = 1.0
    sel_np[0, 192:] = 1.0
    sel_d = nc.inline_tensor(sel_np, "sel_c")

    with tile.TileContext(nc) as tc, ExitStack() as ctx:
        const = ctx.enter_context(tc.tile_pool(name="const", bufs=1))
        ident = const.tile([128, 128], f32, tag="ident")
        m0 = const.tile([128, 64], bf16, tag="m0")
        m1 = const.tile([128, 64], bf16, tag="m1")
        sel = const.tile([1, 256], bf16, tag="sel")
        biasSB = const.tile([1, CHUNK], bf16, tag="bias")
        nc.sync.dma_start(ident[:], ident_d[:])
        nc.sync.dma_start(m0[:], m0_d[:])
        nc.sync.dma_start(m1[:], m1_d[:])
        nc.sync.dma_start(sel[:], sel_d[:])
        nc.sync.dma_start(biasSB[:], biasrow[:])

        big = ctx.enter_context(tc.tile_pool(name="big", bufs=1))
        qT = big.tile([128, SEQ], bf16, tag="qT")      # [(h,d), t]
        kT = big.tile([128, SEQ], bf16, tag="kT")
        ksT = big.tile([128, NSB * BS], bf16, tag="ksT")
        # vaug: per 128-token pair a: [V_h0(64) | 1 | V_h1(64) | 1] = 130 cols
        vaug = big.tile([128, 32 * 130], bf16, tag="vaug")
        # vsaug: strided block pair pr: partitions 0-63 = block 2pr tokens,
        # 64-127 = block 2pr+1 tokens; 130-col groups as above
        vsaug = big.tile([128, 4 * 130], bf16, tag="vsaug")
        # vtail: odd blocks 2a+1 at partition base 0 (tail pieces' lhsT)
        vtail = big.tile([64, 32 * 130], bf16, tag="vtail")
        nc.sync.dma_start(qT[:], qT_d[:])
        nc.sync.dma_start(kT[:], kT_d[:])
        nc.sync.dma_start(ksT[:], ksT_d[:])
        nc.sync.dma_start(vaug[:], vaug_d[:])
        nc.sync.dma_start(vsaug[:], vsaug_d[:])
        nc.sync.dma_start(vtail[:], vtail_d[:])

        # ---- chunked attention ----
        expp = ctx.enter_context(tc.tile_pool(name="expp", bufs=26))
        psS = ctx.enter_context(tc.tile_pool(name="psS", bufs=4, space="PSUM"))
        psOT = ctx.enter_context(tc.tile_pool(name="psOT", bufs=2, space="PSUM"))
        psT = ctx.enter_context(tc.tile_pool(name="psT", bufs=2, space="PSUM"))
        sot = ctx.enter_context(tc.tile_pool(name="sot", bufs=2))
        outp = ctx.enter_context(tc.tile_pool(name="outp", bufs=8))
        rcp = ctx.enter_context(tc.tile_pool(name="rcp", bufs=4))

        for c in (range(NCH) if chunks is None else chunks):
            otiles = [outp.tile([128, 128], f32, tag="ot", name=f"ot{c}_{k4}")
                      for k4 in range(4)]
            for h in heads:
                hq = slice(h * 64, (h + 1) * 64)
                hv = h * 65
                ot = psOT.tile([65, CHUNK], f32, tag="psOT")
                nc.vector.memset(ot[:], 0.0)
                pieces = []  # (expT ap, lhsT ap, out_col, width)

                # strided: packed block b (k-block j = s+8b) valid for chunks
                # c >= b+1; boundary chunk (c == b+1) masks cols < s*64 via a
                # -1e9 bias row on that block's partition half. Blocks paired
                # two-per-matmul on 128 partitions.
                nsb_c = min(c, 7)          # valid strided blocks for chunk c
                for pr in range((nsb_c + 1) // 2):
                    b0 = 2 * pr
                    npart = 128 if (b0 + 1 < nsb_c) else 64
                    bnd = (c - 1) // 2 == pr   # boundary block in this piece
                    ps = psS.tile([128, CHUNK], f32, tag="psS")
                    nc.tensor.matmul(ps[:npart, :],
                                     ksT[hq, b0 * 64:b0 * 64 + npart],
                                     qT[hq, c * CHUNK:(c + 1) * CHUNK],
                                     start=True, stop=not bnd)
                    if bnd:
                        si = 0 if npart == 64 else 128
                        nc.tensor.matmul(ps[:npart, :],
                                         sel[0:1, si:si + npart], biasSB[:],
                                         start=False, stop=True)
                    et = expp.tile([128, CHUNK], bf16, tag="exp")
                    nc.scalar.activation(et[:npart, :], ps[:npart, :], Exp,
                                         scale=SM_SCALE)
                    pieces.append((et[:npart, :],
                                   vsaug[:npart, pr * 130 + hv:pr * 130 + hv + 65],
                                   0, CHUNK))

                # local diagonal column-blocks (masked)
                for a in range(4 * c, 4 * c + 4):
                    for d01 in range(2):
                        i = 2 * a + d01
                        ps = psS.tile([128, 64], f32, tag="psS")
                        nc.tensor.matmul(ps[:], kT[hq, a * 128:(a + 1) * 128],
                                         qT[hq, i * 64:(i + 1) * 64],
                                         start=True, stop=True)
                        et = expp.tile([128, 64], bf16, tag="exp")
                        nc.scalar.activation(et[:], ps[:], Exp, scale=SM_SCALE)
                        nc.vector.tensor_mul(et[:], et[:], (m0 if d01 == 0 else m1)[:])
                        pieces.append((et[:], vaug[:, a * 130 + hv:a * 130 + hv + 65],
                                       (i - 8 * c) * 64, 64))

                # local fully-allowed: k-pair a vs q-blocks [2a+2, 2a+7]
                for a in range(max(0, 4 * c - 3), min(31, 4 * c + 2) + 1):
                    lo = max(2 * a + 2, 8 * c)
                    hi = min(2 * a + 7, 8 * c + 7, 63)
                    if lo > hi:
                        continue
                    w = (hi - lo + 1) * 64
                    ps = psS.tile([128, CHUNK], f32, tag="psS")
                    nc.tensor.matmul(ps[:, :w], kT[hq, a * 128:(a + 1) * 128],
                                     qT[hq, lo * 64:(hi + 1) * 64],
                                     start=True, stop=True)
                    et = expp.tile([128, CHUNK], bf16, tag="exp")
                    nc.scalar.activation(et[:, :w], ps[:, :w], Exp, scale=SM_SCALE)
                    pieces.append((et[:, :w], vaug[:, a * 130 + hv:a * 130 + hv + 65],
                                   (lo - 8 * c) * 64, w))

                # local tails: (j = 2a+1, i = 2a+8), i-j = 7
                for a in range(max(0, 4 * c - 4), 4 * c):
                    i = 2 * a + 8
                    if not (8 * c <= i <= min(8 * c + 7, 63)):
                        continue
                    ps = psS.tile([64, 64], f32, tag="psS")
                    nc.tensor.matmul(ps[:], kT[hq, (2 * a + 1) * 64:(2 * a + 2) * 64],
                                     qT[hq, i * 64:(i + 1) * 64],
                                     start=True, stop=True)
                    et = expp.tile([64, 64], bf16, tag="exp")
                    nc.scalar.activation(et[:], ps[:], Exp, scale=SM_SCALE)
                    pieces.append((et[:], vtail[:, a * 130 + hv:a * 130 + hv + 65],
                                   (i - 8 * c) * 64, 64))

                # PV accumulation onto the memset PSUM tile; pieces touch
                # different column sub-ranges so none may use start=True.
                for pi, (et, vl, col, w) in enumerate(pieces):
                    nc.tensor.matmul(ot[:, col:col + w], vl, et,
                                     start=False, stop=(pi == len(pieces) - 1),
                                     skip_group_check=True)

                so = sot.tile([65, CHUNK], f32, tag="sot")
                nc.vector.tensor_copy(so[:], ot[:])
                for k4 in range(4):
                    pt = psT.tile([128, 65], f32, tag="psT")
                    nc.tensor.transpose(pt[:], so[:, k4 * 128:(k4 + 1) * 128],
                                        ident[:65, :65])
                    rc = rcp.tile([128, 1], f32, tag="rcp")
                    nc.vector.reciprocal(rc[:], pt[:, 64:65])
                    nc.vector.tensor_scalar_mul(
                        otiles[k4][:, h * 64:(h + 1) * 64], pt[:, 0:64], rc[:])

            for k4 in range(4):
                nc.sync.dma_start(
                    outh[c * CHUNK + k4 * 128:c * CHUNK + (k4 + 1) * 128, :],
                    otiles[k4][:])

    return nc


def _in_maps(q, k, v):
    import ml_dtypes
    bf = ml_dtypes.bfloat16
    maps = []
    ones64 = np.ones(64, np.float32)
    for c in range(NCORES):
        heads = [c, c + 8]
        s = (7 - c) % 8
        qT = np.ascontiguousarray(q[:, heads, :].reshape(SEQ, 128).T).astype(bf)
        kT = np.ascontiguousarray(k[:, heads, :].reshape(SEQ, 128).T).astype(bf)
        # packed strided k blocks (7 real + zero pad), transposed
        ksb = np.zeros((NSB * BS, 128), np.float32)
        vsb = np.zeros((NSB, BS, 128), np.float32)
        for b in range(7):
            j = s + 8 * b
            ksb[b * BS:(b + 1) * BS] = k[j * BS:(j + 1) * BS, heads, :].reshape(BS, 128)
            vsb[b] = v[j * BS:(j + 1) * BS, heads, :].reshape(BS, 128)
        ksT = np.ascontiguousarray(ksb.T).astype(bf)
        # vaug [128, 32*130]: pair a, token p -> [V_h0 | 1 | V_h1 | 1]
        vv = v[:, heads, :].reshape(32, 128, 128)   # [a, p, hd]
        vaug = np.ones((128, 32, 130), np.float32)
        vaug[:, :, 0:64] = vv.transpose(1, 0, 2)[:, :, 0:64]
        vaug[:, :, 65:129] = vv.transpose(1, 0, 2)[:, :, 64:128]
        # vsaug [128, 4*130]: pair pr: partitions 0-63 = block 2pr, 64-127 =
        # block 2pr+1
        vsp = vsb.reshape(4, 2, BS, 128).transpose(1, 2, 0, 3).reshape(128, 4, 128)
        vsaug = np.ones((128, 4, 130), np.float32)
        vsaug[:, :, 0:64] = vsp[:, :, 0:64]
        vsaug[:, :, 65:129] = vsp[:, :, 64:128]
        # vtail [64, 32*130]: odd blocks 2a+1
        vt = v[:, heads, :].reshape(32, 2, 64, 128)[:, 1]   # [a, p, hd]
        vtail = np.ones((64, 32, 130), np.float32)
        vtail[:, :, 0:64] = vt.transpose(1, 0, 2)[:, :, 0:64]
        vtail[:, :, 65:129] = vt.transpose(1, 0, 2)[:, :, 64:128]
        bias = np.zeros((1, CHUNK), np.float32)
        bias[0, :s * 64] = -1e9
        maps.append({"qT": qT, "kT": kT, "ksT": ksT,
                     "vaug": vaug.reshape(128, 32 * 130).astype(bf),
                     "vsaug": vsaug.reshape(128, 4 * 130).astype(bf),
                     "vtail": vtail.reshape(64, 32 * 130).astype(bf),
                     "biasrow": bias.astype(bf)})
    return maps


def kernel(q, k, v, cu_seqlens_k=None, **_):
    from concourse.bass_utils import run_bass_kernel_spmd

    q = np.asarray(q, np.float32)
    k = np.asarray(k, np.float32)
    v = np.asarray(v, np.float32)
    if "nc" not in _cache:
        _cache["nc"] = _legalize_waits(_build_program())
    res = run_bass_kernel_spmd(_cache["nc"], _in_maps(q, k, v),
                               list(range(NCORES))).results
    out = np.empty((SEQ, N_HEADS, HEAD), np.float32)
    for c in range(NCORES):
        o = res[c]["outh"].reshape(SEQ, 2, HEAD)
        out[:, c, :] = o[:, 0, :]
        out[:, c + 8, :] = o[:, 1, :]
    return out


# revision 8
# speedup vs baseline: 2.4691x; 1.2294x over previous
"""Block-sparse local+strided attention (LocalStridedBlockSparseAttn) on 8 trn2 cores.

Problem: q,k,v [4096, 16, 64] f32, single prefill sequence. Per-head block mask
(64x64 token blocks): j <= i and (i - j < 8  or  (j + h + 1) % 8 == 0).

Sharding: core c owns heads {c, c+8} - both have the same strided residue
s = (7 - c) % 8, so one SPMD program serves all 8 cores with per-core data.

v3: instruction-count-oriented restructure (v2 was PE-bound on ~1500 tensor
instructions averaging ~400ns fixed cost each):
  - local part per (chunk, head): one [128, w] matmul per k-block PAIR over
    its contiguous valid q-window (w up to 512), masked post-exp with small
    constant masks (m01 for the diagonal 128 cols, mB for the trailing
    64-col block where only the pair's upper k-block is in range), plus one
    64x64 tail piece. 8 pieces replace v2's 15 (diag/full/tail split).
  - small local pieces are packed into shared PSUM banks so ONE activation
    instruction serves several matmuls.
  - strided validity boundary folded into the CONTRACTION: ksT carries 7
    extra indicator partitions (one per packed block) and the strided copy
    of q carries matching -1e9 rows on the boundary-chunk prefix columns,
    so scores = K^T q + bias with zero extra instructions (replaces v2's
    bias matmuls). Per-core data, uniform program.
  - epilogue per (chunk, head): PSUM->SBUF copy on the idle GpSimd engine,
    4 transposes into ONE PSUM tile, a single strided-AP reciprocal, and
    the first full-width PV matmul uses start=True (no memset).
All matmul operands bf16; exp outputs bf16 (PSUM accumulates fp32).
"""

import numpy as np

N_HEADS = 16
HEAD = 64
SEQ = 4096
BS = 64
NB = 64          # 64 token-blocks
LOCAL = 8
VERT = 8
SM_SCALE = 1.0 / 8.0
NCORES = 8
CHUNK = 512      # q tokens per chunk (8 blocks)
NCH = SEQ // CHUNK
NSB = 8          # packed strided block slots (7 real, slot 7 zero pad)
KSP = 64 + 7     # ksT/qs partitions: 64 head dims + 7 boundary indicator rows

_cache = {}


def _legalize_waits(nc, max_waits=1):
    """This walrus build rejects instructions carrying more than one sync-wait
    condition ("Too many sync wait commands"); hoist extras into same-engine
    NoOps placed immediately before the instruction."""
    import concourse.mybir as mybir

    nid = 0
    for bb in nc.main_func.blocks:
        new = []
        for ins in bb.instructions:
            si = ins.sync_info
            if si is not None and si.on_wait and len(si.on_wait) > max_waits:
                waits = list(si.on_wait)
                while len(waits) > max_waits:
                    chunk, waits = waits[:max_waits], waits[max_waits:]
                    nid += 1
                    nop = mybir.InstNoOp(name=f"{ins.name}-wsplit{nid}")
                    nop.engine = ins.engine
                    nop.sync_info = mybir.SyncInfo(on_wait=chunk, on_update=[])
                    new.append(nop)
                ins.sync_info = mybir.SyncInfo(on_wait=waits,
                                               on_update=list(si.on_update))
            new.append(ins)
        bb.instructions[:] = new
    return nc


def _build_program(chunks=None, heads=(0, 1)):
    from contextlib import ExitStack

    import concourse.bass as bass
    import concourse.mybir as mybir
    from concourse import tile

    f32 = mybir.dt.float32
    bf16 = mybir.dt.bfloat16
    Exp = mybir.ActivationFunctionType.Exp

    nc = bass.Bass()
    qT_d = nc.dram_tensor("qT", [128, SEQ], bf16, kind="ExternalInput")
    kT_d = nc.dram_tensor("kT", [128, SEQ], bf16, kind="ExternalInput")
    qs_d = [nc.dram_tensor(f"qs{h}", [KSP, SEQ], bf16, kind="ExternalInput")
            for h in range(2)]
    ks_d = [nc.dram_tensor(f"ks{h}", [KSP, NSB * BS], bf16, kind="ExternalInput")
            for h in range(2)]
    vaug_d = nc.dram_tensor("vaug", [128, 32 * 130], bf16, kind="ExternalInput")
    vsaug_d = nc.dram_tensor("vsaug", [128, 4 * 130], bf16, kind="ExternalInput")
    vtail_d = nc.dram_tensor("vtail", [64, 32 * 130], bf16, kind="ExternalInput")
    outh = nc.dram_tensor("outh", [SEQ, 128], f32, kind="ExternalOutput")

    # Device-constant tiles (same on every core).
    import ml_dtypes
    bf = ml_dtypes.bfloat16
    n = np.arange(64)
    tri = (n[None, :] >= n[:, None]).astype(np.float32)
    m01_np = np.zeros((128, 128), np.float32)
    m01_np[:64, :64] = tri          # q-block 2p vs k-block 2p
    m01_np[:64, 64:] = 1.0          # q-block 2p+1 vs k-block 2p
    m01_np[64:, 64:] = tri          # q-block 2p+1 vs k-block 2p+1
    mB_np = np.zeros((128, 64), np.float32)
    mB_np[64:] = 1.0                # q-block 2p+8: only k-block 2p+1 valid
    ident_d = nc.inline_tensor(np.eye(128, dtype=np.float32), "ident_c")
    m01_d = nc.inline_tensor(m01_np.astype(bf), "m01_c")
    mB_d = nc.inline_tensor(mB_np.astype(bf), "mB_c")

    with tile.TileContext(nc) as tc, ExitStack() as ctx:
        const = ctx.enter_context(tc.tile_pool(name="const", bufs=1))
        ident = const.tile([128, 128], f32, tag="ident")
        m01 = const.tile([128, 128], bf16, tag="m01")
        mB = const.tile([128, 64], bf16, tag="mB")
        nc.sync.dma_start(ident[:], ident_d[:])
        nc.sync.dma_start(m01[:], m01_d[:])
        nc.sync.dma_start(mB[:], mB_d[:])

        big = ctx.enter_context(tc.tile_pool(name="big", bufs=1))
        qT = big.tile([128, SEQ], bf16, tag="qT")      # [(h,d), t]
        kT = big.tile([128, SEQ], bf16, tag="kT")
        qs = [big.tile([KSP, SEQ], bf16, tag=f"qs{h}", name=f"qs{h}")
              for h in range(2)]
        ks = [big.tile([KSP, NSB * BS], bf16, tag=f"ks{h}", name=f"ks{h}")
              for h in range(2)]
        vaug = big.tile([128, 32 * 130], bf16, tag="vaug")
        vsaug = big.tile([128, 4 * 130], bf16, tag="vsaug")
        vtail = big.tile([64, 32 * 130], bf16, tag="vtail")
        nc.sync.dma_start(qT[:], qT_d[:])
        nc.sync.dma_start(kT[:], kT_d[:])
        for h in range(2):
            nc.sync.dma_start(qs[h][:], qs_d[h][:])
            nc.sync.dma_start(ks[h][:], ks_d[h][:])
        nc.sync.dma_start(vaug[:], vaug_d[:])
        nc.sync.dma_start(vsaug[:], vsaug_d[:])
        nc.sync.dma_start(vtail[:], vtail_d[:])

        # ---- chunked attention ----
        expp = ctx.enter_context(tc.tile_pool(name="expp", bufs=12))
        psS = ctx.enter_context(tc.tile_pool(name="psS", bufs=5, space="PSUM"))
        psOT = ctx.enter_context(tc.tile_pool(name="psOT", bufs=2, space="PSUM"))
        psT = ctx.enter_context(tc.tile_pool(name="psT", bufs=1, space="PSUM"))
        sot = ctx.enter_context(tc.tile_pool(name="sot", bufs=2))
        outp = ctx.enter_context(tc.tile_pool(name="outp", bufs=8))
        rcp = ctx.enter_context(tc.tile_pool(name="rcp", bufs=4))

        for c in (range(NCH) if chunks is None else chunks):
            otiles = [outp.tile([128, 128], f32, tag="ot", name=f"ot{c}_{k4}")
                      for k4 in range(4)]
            for h in heads:
                hq = slice(h * 64, (h + 1) * 64)
                hv = h * 65
                ot = psOT.tile([65, CHUNK], f32, tag="psOT")
                pieces = []  # (et ap, vl ap, ot_col, w)

                # strided pairs: full-width pieces, one PSUM bank each; the
                # boundary-chunk prefix masking rides in the contraction
                # (indicator rows of ks x -1e9 rows of qs).
                for pr in range((c + 1) // 2):
                    npart = 128 if 2 * pr + 1 < c else 64
                    ps = psS.tile([128, CHUNK], f32, tag="psS")
                    nc.tensor.matmul(ps[:npart, :],
                                     ks[h][:, pr * 128:pr * 128 + npart],
                                     qs[h][:, c * CHUNK:(c + 1) * CHUNK],
                                     start=True, stop=True)
                    et = expp.tile([128, CHUNK], bf16, tag="exp")
                    nc.scalar.activation(et[:npart, :], ps[:npart, :], Exp,
                                         scale=SM_SCALE)
                    pieces.append((et[:npart, :],
                                   vsaug[:npart, pr * 130 + hv:pr * 130 + hv + 65],
                                   0, CHUNK))

                # local: one piece per k-pair window (+ tail), packed into
                # shared PSUM banks so one exp serves several matmuls.
                locs = []  # (p, wlo, whi, npart, kind)
                for p in range(max(0, 4 * c - 4), min(31, 4 * c + 3) + 1):
                    if p == 4 * c - 4:
                        locs.append((p, 8 * c, 8 * c, 64, 2))     # tail
                        continue
                    wlo = max(8 * c, 2 * p)
                    whi = min(8 * c + 7, 2 * p + 8, 63)
                    if wlo > whi:
                        continue
                    locs.append((p, wlo, whi, 128, 0 if p >= 4 * c else 1))
                # first-fit-decreasing packing into 512-col banks
                banks = []  # [used, [(p, wlo, whi, npart, kind, off), ...]]
                for ent in sorted(locs, key=lambda e: -(e[2] - e[1] + 1)):
                    w = (ent[2] - ent[1] + 1) * 64
                    for bk in banks:
                        if bk[0] + w <= CHUNK:
                            bk[1].append(ent + (bk[0],))
                            bk[0] += w
                            break
                    else:
                        banks.append([w, [ent + (0,)]])
                for used, subs in banks:
                    ps = psS.tile([128, CHUNK], f32, tag="psS")
                    for p, wlo, whi, npart, kind, off in subs:
                        w = (whi - wlo + 1) * 64
                        if kind == 2:   # tail: single k-block 2p+1 vs i=8c
                            lhs = kT[hq, (2 * p + 1) * 64:(2 * p + 2) * 64]
                        else:
                            lhs = kT[hq, 2 * p * 64:2 * p * 64 + 128]
                        nc.tensor.matmul(ps[:npart, off:off + w], lhs,
                                         qT[hq, wlo * 64:(whi + 1) * 64],
                                         start=True, stop=True,
                                         skip_group_check=True)
                    et = expp.tile([128, CHUNK], bf16, tag="exp")
                    nc.scalar.activation(et[:, :used], ps[:, :used], Exp,
                                         scale=SM_SCALE)
                    for p, wlo, whi, npart, kind, off in subs:
                        w = (whi - wlo + 1) * 64
                        if kind == 0:
                            nc.vector.tensor_mul(et[:, off:off + 128],
                                                 et[:, off:off + 128], m01[:])
                        elif kind == 1:
                            nc.vector.tensor_mul(et[:, off + w - 64:off + w],
                                                 et[:, off + w - 64:off + w],
                                                 mB[:])
                        vl = (vtail if kind == 2 else vaug)[
                            :npart, p * 130 + hv:p * 130 + hv + 65]
                        pieces.append((et[:npart, off:off + w], vl,
                                       (wlo - 8 * c) * 64, w))

                # PV accumulation; first piece must cover the full 512 cols
                # (start=True replaces a memset), the rest accumulate.
                ffull = next(i for i, pc in enumerate(pieces) if pc[3] == CHUNK)
                pieces[0], pieces[ffull] = pieces[ffull], pieces[0]
                for pi, (et, vl, col, w) in enumerate(pieces):
                    nc.tensor.matmul(ot[:, col:col + w], vl, et,
                                     start=(pi == 0), stop=(pi == len(pieces) - 1),
                                     skip_group_check=True)

                so = sot.tile([65, CHUNK], f32, tag="sot")
                nc.vector.tensor_copy(so[:], ot[:])
                pt = psT.tile([128, 4 * 65], f32, tag="psT")
                for k4 in range(4):
                    nc.tensor.matmul(pt[:, k4 * 65:(k4 + 1) * 65],
                                     so[:, k4 * 128:(k4 + 1) * 128],
                                     ident[:65, :65], is_transpose=True,
                                     skip_group_check=True)
                rc = rcp.tile([128, 4], f32, tag="rcp")
                nc.vector.reciprocal(
                    rc[:].rearrange("p (k c) -> p k c", c=1),
                    pt[:].rearrange("p (k c) -> p k c", c=65)[:, :, 64:65])
                for k4 in range(4):
                    nc.vector.tensor_scalar_mul(
                        otiles[k4][:, h * 64:(h + 1) * 64],
                        pt[:, k4 * 65:k4 * 65 + 64], rc[:, k4:k4 + 1])

            for k4 in range(4):
                nc.sync.dma_start(
                    outh[c * CHUNK + k4 * 128:c * CHUNK + (k4 + 1) * 128, :],
                    otiles[k4][:])

    return nc


def _in_maps(q, k, v):
    import ml_dtypes
    bf = ml_dtypes.bfloat16
    maps = []
    for c in range(NCORES):
        heads = [c, c + 8]
        s = (7 - c) % 8
        qT = np.ascontiguousarray(q[:, heads, :].reshape(SEQ, 128).T).astype(bf)
        kT = np.ascontiguousarray(k[:, heads, :].reshape(SEQ, 128).T).astype(bf)
        # strided contraction operands with boundary-bias augmentation:
        # ks[h] rows 64+b indicate packed block b's columns; qs[h] rows 64+b
        # carry -1e9 on chunk b+1's first s*64 columns.
        ksb = np.zeros((NSB * BS, 128), np.float32)
        vsb = np.zeros((NSB, BS, 128), np.float32)
        for b in range(7):
            j = s + 8 * b
            ksb[b * BS:(b + 1) * BS] = k[j * BS:(j + 1) * BS, heads, :].reshape(BS, 128)
            vsb[b] = v[j * BS:(j + 1) * BS, heads, :].reshape(BS, 128)
        ind = np.zeros((7, NSB * BS), np.float32)
        for b in range(7):
            ind[b, b * BS:(b + 1) * BS] = 1.0
        wrow = np.zeros((7, SEQ), np.float32)
        for b in range(7):
            wrow[b, (b + 1) * CHUNK:(b + 1) * CHUNK + s * 64] = -1e9
        qsl, ksl = [], []
        for hh in range(2):
            qs_h = np.concatenate(
                [q[:, heads[hh], :].T.astype(np.float32), wrow], axis=0)
            ks_h = np.concatenate(
                [ksb[:, hh * 64:(hh + 1) * 64].T, ind], axis=0)
            qsl.append(np.ascontiguousarray(qs_h).astype(bf))
            ksl.append(np.ascontiguousarray(ks_h).astype(bf))
        # vaug [128, 32*130]: pair a, token p -> [V_h0 | 1 | V_h1 | 1]
        vv = v[:, heads, :].reshape(32, 128, 128)   # [a, p, hd]
        vaug = np.ones((128, 32, 130), np.float32)
        vaug[:, :, 0:64] = vv.transpose(1, 0, 2)[:, :, 0:64]
        vaug[:, :, 65:129] = vv.transpose(1, 0, 2)[:, :, 64:128]
        # vsaug [128, 4*130]: pair pr: partitions 0-63 = block 2pr, 64-127 =
        # block 2pr+1
        vsp = vsb.reshape(4, 2, BS, 128).transpose(1, 2, 0, 3).reshape(128, 4, 128)
        vsaug = np.ones((128, 4, 130), np.float32)
        vsaug[:, :, 0:64] = vsp[:, :, 0:64]
        vsaug[:, :, 65:129] = vsp[:, :, 64:128]
        # vtail [64, 32*130]: odd blocks 2a+1
        vt = v[:, heads, :].reshape(32, 2, 64, 128)[:, 1]   # [a, p, hd]
        vtail = np.ones((64, 32, 130), np.float32)
        vtail[:, :, 0:64] = vt.transpose(1, 0, 2)[:, :, 0:64]
        vtail[:, :, 65:129] = vt.transpose(1, 0, 2)[:, :, 64:128]
        maps.append({"qT": qT, "kT": kT,
                     "qs0": qsl[0], "qs1": qsl[1],
                     "ks0": ksl[0], "ks1": ksl[1],
                     "vaug": vaug.reshape(128, 32 * 130).astype(bf),
                     "vsaug": vsaug.reshape(128, 4 * 130).astype(bf),
                     "vtail": vtail.reshape(64, 32 * 130).astype(bf)})
    return maps


def kernel(q, k, v, cu_seqlens_k=None, **_):
    from concourse.bass_utils import run_bass_kernel_spmd

    q = np.asarray(q, np.float32)
    k = np.asarray(k, np.float32)
    v = np.asarray(v, np.float32)
    if "nc" not in _cache:
        _cache["nc"] = _legalize_waits(_build_program())
    res = run_bass_kernel_spmd(_cache["nc"], _in_maps(q, k, v),
                               list(range(NCORES))).results
    out = np.empty((SEQ, N_HEADS, HEAD), np.float32)
    for c in range(NCORES):
        o = res[c]["outh"].reshape(SEQ, 2, HEAD)
        out[:, c, :] = o[:, 0, :]
        out[:, c + 8, :] = o[:, 1, :]
    return out


# revision 11
# speedup vs baseline: 2.8243x; 1.1438x over previous
"""Block-sparse local+strided attention (LocalStridedBlockSparseAttn) on 8 trn2 cores.

Problem: q,k,v [4096, 16, 64] f32, single prefill sequence. Per-head block mask
(64x64 token blocks): j <= i and (i - j < 8  or  (j + h + 1) % 8 == 0).

Sharding: core c owns heads {c, c+8} - both have the same strided residue
s = (7 - c) % 8, so one SPMD program serves all 8 cores with per-core data.

v4 (instruction-minimal dataflow; v3 was PE-bound on per-instruction fixed
costs and startup DMA serialization):
  - local part per (chunk, head): one [128, w] matmul per k-block PAIR over
    its contiguous valid q-window (w up to 512), masked post-exp with small
    constant masks, plus one 64x64 tail piece; small pieces packed into
    shared PSUM banks so ONE activation serves several matmuls.
  - strided validity boundary folded into the CONTRACTION: ks carries 7
    indicator partitions and the strided q copy carries -1e9 rows on the
    boundary-chunk prefix columns (zero per-piece instructions).
  - output stays TRANSPOSED with the rowsums row: the [65, 512] PSUM tile
    [O^T; rowsums] is DMA'd straight to DRAM; the host does the divide and
    the final transpose. No PE transposes, no reciprocal/normalize/copy
    instructions on device.
  - big inputs split in halves with chunk-0-critical slices DMA'd first so
    compute starts before the tail of the input load.
All matmul operands bf16; exp outputs bf16 (PSUM accumulates fp32).
"""

import numpy as np

N_HEADS = 16
HEAD = 64
SEQ = 4096
BS = 64
NB = 64          # 64 token-blocks
LOCAL = 8
VERT = 8
SM_SCALE = 1.0 / 8.0
NCORES = 8
CHUNK = 512      # q tokens per chunk (8 blocks)
NCH = SEQ // CHUNK
NSB = 8          # packed strided block slots (7 real, slot 7 zero pad)
KSP = 64 + 7     # ks/qs partitions: 64 head dims + 7 boundary indicator rows
HSEQ = SEQ // 2

_cache = {}


def _legalize_waits(nc, max_waits=1):
    """This walrus build rejects instructions carrying more than one sync-wait
    condition ("Too many sync wait commands"); hoist extras into same-engine
    NoOps placed immediately before the instruction."""
    import concourse.mybir as mybir

    nid = 0
    for bb in nc.main_func.blocks:
        new = []
        for ins in bb.instructions:
            si = ins.sync_info
            if si is not None and si.on_wait and len(si.on_wait) > max_waits:
                waits = list(si.on_wait)
                while len(waits) > max_waits:
                    chunk, waits = waits[:max_waits], waits[max_waits:]
                    nid += 1
                    nop = mybir.InstNoOp(name=f"{ins.name}-wsplit{nid}")
                    nop.engine = ins.engine
                    nop.sync_info = mybir.SyncInfo(on_wait=chunk, on_update=[])
                    new.append(nop)
                ins.sync_info = mybir.SyncInfo(on_wait=waits,
                                               on_update=list(si.on_update))
            new.append(ins)
        bb.instructions[:] = new
    return nc


def _build_program(chunks=None, heads=(0, 1)):
    from contextlib import ExitStack

    import concourse.bass as bass
    import concourse.mybir as mybir
    from concourse import tile

    f32 = mybir.dt.float32
    bf16 = mybir.dt.bfloat16
    Exp = mybir.ActivationFunctionType.Exp

    nc = bass.Bass()
    qT_d = nc.dram_tensor("qT", [128, SEQ], bf16, kind="ExternalInput")
    kT_d = nc.dram_tensor("kT", [128, SEQ], bf16, kind="ExternalInput")
    qs_d = [nc.dram_tensor(f"qs{h}", [KSP, SEQ], bf16, kind="ExternalInput")
            for h in range(2)]
    ks_d = [nc.dram_tensor(f"ks{h}", [KSP, NSB * BS], bf16, kind="ExternalInput")
            for h in range(2)]
    vaug_d = nc.dram_tensor("vaug", [128, 32 * 130], bf16, kind="ExternalInput")
    vsaug_d = nc.dram_tensor("vsaug", [128, 4 * 130], bf16, kind="ExternalInput")
    vtail_d = nc.dram_tensor("vtail", [64, 32 * 130], bf16, kind="ExternalInput")
    # transposed output with rowsums: rows h*65..h*65+63 = O^T, row h*65+64 =
    # softmax denominators; host divides + transposes back.
    outT_d = nc.dram_tensor("outT", [130, SEQ], f32, kind="ExternalOutput")

    # Device-constant tiles (same on every core).
    import ml_dtypes
    bf = ml_dtypes.bfloat16
    n = np.arange(64)
    tri = (n[None, :] >= n[:, None]).astype(np.float32)
    m01_np = np.zeros((128, 128), np.float32)
    m01_np[:64, :64] = tri          # q-block 2p vs k-block 2p
    m01_np[:64, 64:] = 1.0          # q-block 2p+1 vs k-block 2p
    m01_np[64:, 64:] = tri          # q-block 2p+1 vs k-block 2p+1
    mB_np = np.zeros((128, 64), np.float32)
    mB_np[64:] = 1.0                # q-block 2p+8: only k-block 2p+1 valid
    m01_d = nc.inline_tensor(m01_np.astype(bf), "m01_c")
    mB_d = nc.inline_tensor(mB_np.astype(bf), "mB_c")

    with tile.TileContext(nc) as tc, ExitStack() as ctx:
        const = ctx.enter_context(tc.tile_pool(name="const", bufs=1))
        m01 = const.tile([128, 128], bf16, tag="m01")
        mB = const.tile([128, 64], bf16, tag="mB")
        nc.sync.dma_start(m01[:], m01_d[:])
        nc.sync.dma_start(mB[:], mB_d[:])

        big = ctx.enter_context(tc.tile_pool(name="big", bufs=1))
        # halves: [cols 0..2047], [cols 2048..4095] so chunk 0 starts early
        kTh = [big.tile([128, HSEQ], bf16, tag=f"kT{i}", name=f"kT{i}")
               for i in range(2)]
        qTh = [big.tile([128, HSEQ], bf16, tag=f"qT{i}", name=f"qT{i}")
               for i in range(2)]
        qsh = [[big.tile([KSP, HSEQ], bf16, tag=f"qs{h}_{i}", name=f"qs{h}_{i}")
                for i in range(2)] for h in range(2)]
        ks = [big.tile([KSP, NSB * BS], bf16, tag=f"ks{h}", name=f"ks{h}")
              for h in range(2)]
        vaugh = [big.tile([128, 16 * 130], bf16, tag=f"vaug{i}", name=f"vaug{i}")
                 for i in range(2)]
        vsaug = big.tile([128, 4 * 130], bf16, tag="vsaug")
        vtailh = [big.tile([64, 16 * 130], bf16, tag=f"vtail{i}", name=f"vtail{i}")
                  for i in range(2)]
        # chunk-0-critical first, then the rest
        nc.sync.dma_start(kTh[0][:], kT_d[:, 0:HSEQ])
        nc.sync.dma_start(qTh[0][:], qT_d[:, 0:HSEQ])
        nc.sync.dma_start(vaugh[0][:], vaug_d[:, 0:16 * 130])
        nc.sync.dma_start(vtailh[0][:], vtail_d[:, 0:16 * 130])
        for h in range(2):
            nc.sync.dma_start(ks[h][:], ks_d[h][:])
        nc.sync.dma_start(vsaug[:], vsaug_d[:])
        for h in range(2):
            nc.sync.dma_start(qsh[h][0][:], qs_d[h][:, 0:HSEQ])
        nc.sync.dma_start(kTh[1][:], kT_d[:, HSEQ:SEQ])
        nc.sync.dma_start(qTh[1][:], qT_d[:, HSEQ:SEQ])
        nc.sync.dma_start(vaugh[1][:], vaug_d[:, 16 * 130:32 * 130])
        nc.sync.dma_start(vtailh[1][:], vtail_d[:, 16 * 130:32 * 130])
        for h in range(2):
            nc.sync.dma_start(qsh[h][1][:], qs_d[h][:, HSEQ:SEQ])

        def kT_ap(hq, col, w):
            i, base = (0, 0) if col < HSEQ else (1, HSEQ)
            return kTh[i][hq, col - base:col - base + w]

        def qT_ap(hq, col, w):
            i, base = (0, 0) if col < HSEQ else (1, HSEQ)
            return qTh[i][hq, col - base:col - base + w]

        def vaug_ap(np_, p, off, w):
            i, base = (0, 0) if p < 16 else (1, 16)
            return vaugh[i][:np_, (p - base) * 130 + off:(p - base) * 130 + off + w]

        def vtail_ap(np_, p, off, w):
            i, base = (0, 0) if p < 16 else (1, 16)
            return vtailh[i][:np_, (p - base) * 130 + off:(p - base) * 130 + off + w]

        # ---- chunked attention ----
        expp = ctx.enter_context(tc.tile_pool(name="expp", bufs=12))
        psS = ctx.enter_context(tc.tile_pool(name="psS", bufs=5, space="PSUM"))
        psOT = ctx.enter_context(tc.tile_pool(name="psOT", bufs=3, space="PSUM"))
        sot = ctx.enter_context(tc.tile_pool(name="sot", bufs=3))

        for c in (range(NCH) if chunks is None else chunks):
            for h in heads:
                hq = slice(h * 64, (h + 1) * 64)
                hv = h * 65
                ot = psOT.tile([65, CHUNK], f32, tag="psOT")
                pieces = []  # (et ap, vl ap, ot_col, w)

                # strided pairs: full-width pieces, one PSUM bank each; the
                # boundary-chunk prefix masking rides in the contraction
                # (indicator rows of ks x -1e9 rows of qs).
                qschunk = qsh[h][0 if c < 4 else 1][
                    :, (c % 4) * CHUNK:(c % 4 + 1) * CHUNK]
                for pr in range((c + 1) // 2):
                    npart = 128 if 2 * pr + 1 < c else 64
                    ps = psS.tile([128, CHUNK], f32, tag="psS")
                    nc.tensor.matmul(ps[:npart, :],
                                     ks[h][:, pr * 128:pr * 128 + npart],
                                     qschunk, start=True, stop=True)
                    et = expp.tile([128, CHUNK], bf16, tag="exp")
                    nc.scalar.activation(et[:npart, :], ps[:npart, :], Exp,
                                         scale=SM_SCALE)
                    pieces.append((et[:npart, :],
                                   vsaug[:npart, pr * 130 + hv:pr * 130 + hv + 65],
                                   0, CHUNK))

                # local: one piece per k-pair window (+ tail), packed into
                # shared PSUM banks so one exp serves several matmuls.
                locs = []  # (p, wlo, whi, npart, kind)
                for p in range(max(0, 4 * c - 4), min(31, 4 * c + 3) + 1):
                    if p == 4 * c - 4:
                        locs.append((p, 8 * c, 8 * c, 64, 2))     # tail
                        continue
                    wlo = max(8 * c, 2 * p)
                    whi = min(8 * c + 7, 2 * p + 8, 63)
                    if wlo > whi:
                        continue
                    locs.append((p, wlo, whi, 128, 0 if p >= 4 * c else 1))
                # first-fit-decreasing packing into 512-col banks
                banks = []  # [used, [(p, wlo, whi, npart, kind, off), ...]]
                for ent in sorted(locs, key=lambda e: -(e[2] - e[1] + 1)):
                    w = (ent[2] - ent[1] + 1) * 64
                    for bk in banks:
                        if bk[0] + w <= CHUNK:
                            bk[1].append(ent + (bk[0],))
                            bk[0] += w
                            break
                    else:
                        banks.append([w, [ent + (0,)]])
                for used, subs in banks:
                    ps = psS.tile([128, CHUNK], f32, tag="psS")
                    for p, wlo, whi, npart, kind, off in subs:
                        w = (whi - wlo + 1) * 64
                        if kind == 2:   # tail: single k-block 2p+1 vs i=8c
                            lhs = kT_ap(hq, (2 * p + 1) * 64, 64)
                        else:
                            lhs = kT_ap(hq, 2 * p * 64, 128)
                        nc.tensor.matmul(ps[:npart, off:off + w], lhs,
                                         qT_ap(hq, wlo * 64, w),
                                         start=True, stop=True,
                                         skip_group_check=True)
                    et = expp.tile([128, CHUNK], bf16, tag="exp")
                    nc.scalar.activation(et[:, :used], ps[:, :used], Exp,
                                         scale=SM_SCALE)
                    for p, wlo, whi, npart, kind, off in subs:
                        w = (whi - wlo + 1) * 64
                        if kind == 0:
                            nc.vector.tensor_mul(et[:, off:off + 128],
                                                 et[:, off:off + 128], m01[:])
                        elif kind == 1:
                            nc.vector.tensor_mul(et[:, off + w - 64:off + w],
                                                 et[:, off + w - 64:off + w],
                                                 mB[:])
                        if kind == 2:
                            vl = vtail_ap(npart, p, hv, 65)
                        else:
                            vl = vaug_ap(npart, p, hv, 65)
                        pieces.append((et[:npart, off:off + w], vl,
                                       (wlo - 8 * c) * 64, w))

                # PV accumulation; first piece must cover the full 512 cols
                # (start=True replaces a memset), the rest accumulate.
                ffull = next(i for i, pc in enumerate(pieces) if pc[3] == CHUNK)
                pieces[0], pieces[ffull] = pieces[ffull], pieces[0]
                for pi, (et, vl, col, w) in enumerate(pieces):
                    nc.tensor.matmul(ot[:, col:col + w], vl, et,
                                     start=(pi == 0), stop=(pi == len(pieces) - 1),
                                     skip_group_check=True)

                # [O^T; rowsums] to DRAM via SBUF staging; host normalizes.
                so = sot.tile([65, CHUNK], f32, tag="sot")
                nc.vector.tensor_copy(so[:], ot[:])
                nc.sync.dma_start(
                    outT_d[h * 65:(h + 1) * 65, c * CHUNK:(c + 1) * CHUNK],
                    so[:])

    return nc


def _in_maps(q, k, v):
    import ml_dtypes
    bf = ml_dtypes.bfloat16
    maps = []
    for c in range(NCORES):
        heads = [c, c + 8]
        s = (7 - c) % 8
        qT = np.ascontiguousarray(q[:, heads, :].reshape(SEQ, 128).T).astype(bf)
        kT = np.ascontiguousarray(k[:, heads, :].reshape(SEQ, 128).T).astype(bf)
        # strided contraction operands with boundary-bias augmentation:
        # ks[h] rows 64+b indicate packed block b's columns; qs[h] rows 64+b
        # carry -1e9 on chunk b+1's first s*64 columns.
        ksb = np.zeros((NSB * BS, 128), np.float32)
        vsb = np.zeros((NSB, BS, 128), np.float32)
        for b in range(7):
            j = s + 8 * b
            ksb[b * BS:(b + 1) * BS] = k[j * BS:(j + 1) * BS, heads, :].reshape(BS, 128)
            vsb[b] = v[j * BS:(j + 1) * BS, heads, :].reshape(BS, 128)
        ind = np.zeros((7, NSB * BS), np.float32)
        for b in range(7):
            ind[b, b * BS:(b + 1) * BS] = 1.0
        wrow = np.zeros((7, SEQ), np.float32)
        for b in range(7):
            wrow[b, (b + 1) * CHUNK:(b + 1) * CHUNK + s * 64] = -1e9
        qsl, ksl = [], []
        for hh in range(2):
            qs_h = np.concatenate(
                [q[:, heads[hh], :].T.astype(np.float32), wrow], axis=0)
            ks_h = np.concatenate(
                [ksb[:, hh * 64:(hh + 1) * 64].T, ind], axis=0)
            qsl.append(np.ascontiguousarray(qs_h).astype(bf))
            ksl.append(np.ascontiguousarray(ks_h).astype(bf))
        # vaug [128, 32*130]: pair a, token p -> [V_h0 | 1 | V_h1 | 1]
        vv = v[:, heads, :].reshape(32, 128, 128)   # [a, p, hd]
        vaug = np.ones((128, 32, 130), np.float32)
        vaug[:, :, 0:64] = vv.transpose(1, 0, 2)[:, :, 0:64]
        vaug[:, :, 65:129] = vv.transpose(1, 0, 2)[:, :, 64:128]
        # vsaug [128, 4*130]: pair pr: partitions 0-63 = block 2pr, 64-127 =
        # block 2pr+1
        vsp = vsb.reshape(4, 2, BS, 128).transpose(1, 2, 0, 3).reshape(128, 4, 128)
        vsaug = np.ones((128, 4, 130), np.float32)
        vsaug[:, :, 0:64] = vsp[:, :, 0:64]
        vsaug[:, :, 65:129] = vsp[:, :, 64:128]
        # vtail [64, 32*130]: odd blocks 2a+1
        vt = v[:, heads, :].reshape(32, 2, 64, 128)[:, 1]   # [a, p, hd]
        vtail = np.ones((64, 32, 130), np.float32)
        vtail[:, :, 0:64] = vt.transpose(1, 0, 2)[:, :, 0:64]
        vtail[:, :, 65:129] = vt.transpose(1, 0, 2)[:, :, 64:128]
        maps.append({"qT": qT, "kT": kT,
                     "qs0": qsl[0], "qs1": qsl[1],
                     "ks0": ksl[0], "ks1": ksl[1],
                     "vaug": vaug.reshape(128, 32 * 130).astype(bf),
                     "vsaug": vsaug.reshape(128, 4 * 130).astype(bf),
                     "vtail": vtail.reshape(64, 32 * 130).astype(bf)})
    return maps


def kernel(q, k, v, cu_seqlens_k=None, **_):
    from concourse.bass_utils import run_bass_kernel_spmd

    q = np.asarray(q, np.float32)
    k = np.asarray(k, np.float32)
    v = np.asarray(v, np.float32)
    if "nc" not in _cache:
        _cache["nc"] = _legalize_waits(_build_program())
    res = run_bass_kernel_spmd(_cache["nc"], _in_maps(q, k, v),
                               list(range(NCORES))).results
    out = np.empty((SEQ, N_HEADS, HEAD), np.float32)
    for c in range(NCORES):
        o = res[c]["outT"]                      # [130, SEQ]
        for hh, head in ((0, c), (1, c + 8)):
            num = o[hh * 65:hh * 65 + 64, :]    # [64, SEQ]
            den = o[hh * 65 + 64, :]            # [SEQ]
            out[:, head, :] = (num / den).T
    return out


# revision 14
# speedup vs baseline: 2.8616x; 1.0132x over previous
"""Block-sparse local+strided attention (LocalStridedBlockSparseAttn) on 8 trn2 cores.

Problem: q,k,v [4096, 16, 64] f32, single prefill sequence. Per-head block mask
(64x64 token blocks): j <= i and (i - j < 8  or  (j + h + 1) % 8 == 0).

Sharding: core c owns heads {c, c+8} - both have the same strided residue
s = (7 - c) % 8, so one SPMD program serves all 8 cores with per-core data.

v4 (instruction-minimal dataflow; v3 was PE-bound on per-instruction fixed
costs and startup DMA serialization):
  - local part per (chunk, head): one [128, w] matmul per k-block PAIR over
    its contiguous valid q-window (w up to 512), masked post-exp with small
    constant masks, plus one 64x64 tail piece; small pieces packed into
    shared PSUM banks so ONE activation serves several matmuls.
  - strided validity boundary folded into the CONTRACTION: ks carries 7
    indicator partitions and the strided q copy carries -1e9 rows on the
    boundary-chunk prefix columns (zero per-piece instructions).
  - output stays TRANSPOSED with the rowsums row: the [65, 512] PSUM tile
    [O^T; rowsums] is DMA'd straight to DRAM; the host does the divide and
    the final transpose. No PE transposes, no reciprocal/normalize/copy
    instructions on device.
  - big inputs split in halves with chunk-0-critical slices DMA'd first so
    compute starts before the tail of the input load.
All matmul operands bf16; exp outputs bf16 (PSUM accumulates fp32).
"""

import numpy as np

N_HEADS = 16
HEAD = 64
SEQ = 4096
BS = 64
NB = 64          # 64 token-blocks
LOCAL = 8
VERT = 8
SM_SCALE = 1.0 / 8.0
NCORES = 8
CHUNK = 512      # q tokens per chunk (8 blocks)
NCH = SEQ // CHUNK
NSB = 8          # packed strided block slots (7 real, slot 7 zero pad)
KSP = 64 + 7     # ks/qs partitions: 64 head dims + 7 boundary indicator rows
HSEQ = SEQ // 2

_cache = {}


def _legalize_waits(nc, max_waits=1):
    """This walrus build rejects instructions carrying more than one sync-wait
    condition ("Too many sync wait commands"); hoist extras into same-engine
    NoOps placed immediately before the instruction."""
    import concourse.mybir as mybir

    nid = 0
    for bb in nc.main_func.blocks:
        new = []
        for ins in bb.instructions:
            si = ins.sync_info
            if si is not None and si.on_wait and len(si.on_wait) > max_waits:
                waits = list(si.on_wait)
                while len(waits) > max_waits:
                    chunk, waits = waits[:max_waits], waits[max_waits:]
                    nid += 1
                    nop = mybir.InstNoOp(name=f"{ins.name}-wsplit{nid}")
                    nop.engine = ins.engine
                    nop.sync_info = mybir.SyncInfo(on_wait=chunk, on_update=[])
                    new.append(nop)
                ins.sync_info = mybir.SyncInfo(on_wait=waits,
                                               on_update=list(si.on_update))
            new.append(ins)
        bb.instructions[:] = new
    return nc


def _build_program(chunks=None, heads=(0, 1)):
    from contextlib import ExitStack

    import concourse.bass as bass
    import concourse.mybir as mybir
    from concourse import tile

    f32 = mybir.dt.float32
    bf16 = mybir.dt.bfloat16
    Exp = mybir.ActivationFunctionType.Exp

    nc = bass.Bass()
    qT_d = nc.dram_tensor("qT", [128, SEQ], bf16, kind="ExternalInput")
    kT_d = nc.dram_tensor("kT", [128, SEQ], bf16, kind="ExternalInput")
    qs_d = [nc.dram_tensor(f"qs{h}", [KSP, SEQ], bf16, kind="ExternalInput")
            for h in range(2)]
    ks_d = [nc.dram_tensor(f"ks{h}", [KSP, NSB * BS], bf16, kind="ExternalInput")
            for h in range(2)]
    vaug_d = nc.dram_tensor("vaug", [128, 32 * 130], bf16, kind="ExternalInput")
    vsaug_d = nc.dram_tensor("vsaug", [128, 4 * 130], bf16, kind="ExternalInput")
    vtail_d = nc.dram_tensor("vtail", [64, 32 * 130], bf16, kind="ExternalInput")
    # transposed output with rowsums: rows h*65..h*65+63 = O^T, row h*65+64 =
    # softmax denominators; host divides + transposes back.
    outT_d = nc.dram_tensor("outT", [130, SEQ], f32, kind="ExternalOutput")

    # Device-constant tiles (same on every core).
    import ml_dtypes
    bf = ml_dtypes.bfloat16
    n = np.arange(64)
    tri = (n[None, :] >= n[:, None]).astype(np.float32)
    m01_np = np.zeros((128, 128), np.float32)
    m01_np[:64, :64] = tri          # q-block 2p vs k-block 2p
    m01_np[:64, 64:] = 1.0          # q-block 2p+1 vs k-block 2p
    m01_np[64:, 64:] = tri          # q-block 2p+1 vs k-block 2p+1
    mB_np = np.zeros((128, 64), np.float32)
    mB_np[64:] = 1.0                # q-block 2p+8: only k-block 2p+1 valid
    m01_d = nc.inline_tensor(m01_np.astype(bf), "m01_c")
    mB_d = nc.inline_tensor(mB_np.astype(bf), "mB_c")

    with tile.TileContext(nc) as tc, ExitStack() as ctx:
        const = ctx.enter_context(tc.tile_pool(name="const", bufs=1))
        m01 = const.tile([128, 128], bf16, tag="m01")
        mB = const.tile([128, 64], bf16, tag="mB")
        nc.sync.dma_start(m01[:], m01_d[:])
        nc.sync.dma_start(mB[:], mB_d[:])

        big = ctx.enter_context(tc.tile_pool(name="big", bufs=1))
        # quarters (1024 cols) so chunk 0 starts after ~1/4 of the load;
        # issue on BOTH hwdge queues (sync + scalar) in need-order.
        kTq = [big.tile([128, 1024], bf16, tag=f"kT{i}", name=f"kT{i}")
               for i in range(4)]
        qTq = [big.tile([128, 1024], bf16, tag=f"qT{i}", name=f"qT{i}")
               for i in range(4)]
        qsh = [[big.tile([KSP, HSEQ], bf16, tag=f"qs{h}_{i}", name=f"qs{h}_{i}")
                for i in range(2)] for h in range(2)]
        ks = [big.tile([KSP, NSB * BS], bf16, tag=f"ks{h}", name=f"ks{h}")
              for h in range(2)]
        vaugh = [big.tile([128, 16 * 130], bf16, tag=f"vaug{i}", name=f"vaug{i}")
                 for i in range(2)]
        vsaug = big.tile([128, 4 * 130], bf16, tag="vsaug")
        vtailh = [big.tile([64, 16 * 130], bf16, tag=f"vtail{i}", name=f"vtail{i}")
                  for i in range(2)]
        # sync queue: chunk-0-critical k/q slices then the later quarters
        nc.sync.dma_start(kTq[0][:], kT_d[:, 0:1024])
        nc.sync.dma_start(qTq[0][:], qT_d[:, 0:1024])
        nc.sync.dma_start(kTq[1][:], kT_d[:, 1024:2048])
        nc.sync.dma_start(qTq[1][:], qT_d[:, 1024:2048])
        nc.sync.dma_start(kTq[2][:], kT_d[:, 2048:3072])
        nc.sync.dma_start(qTq[2][:], qT_d[:, 2048:3072])
        nc.sync.dma_start(kTq[3][:], kT_d[:, 3072:SEQ])
        nc.sync.dma_start(qTq[3][:], qT_d[:, 3072:SEQ])
        # scalar queue: V layouts + strided operands (ACT is idle at start)
        nc.scalar.dma_start(vaugh[0][:], vaug_d[:, 0:16 * 130])
        nc.scalar.dma_start(vtailh[0][:], vtail_d[:, 0:16 * 130])
        for h in range(2):
            nc.scalar.dma_start(ks[h][:], ks_d[h][:])
        nc.scalar.dma_start(vsaug[:], vsaug_d[:])
        for h in range(2):
            nc.scalar.dma_start(qsh[h][0][:], qs_d[h][:, 0:HSEQ])
        nc.scalar.dma_start(vaugh[1][:], vaug_d[:, 16 * 130:32 * 130])
        nc.scalar.dma_start(vtailh[1][:], vtail_d[:, 16 * 130:32 * 130])
        for h in range(2):
            nc.scalar.dma_start(qsh[h][1][:], qs_d[h][:, HSEQ:SEQ])

        def kT_ap(hq, col, w):
            return kTq[col // 1024][hq, col % 1024:col % 1024 + w]

        def qT_ap(hq, col, w):
            return qTq[col // 1024][hq, col % 1024:col % 1024 + w]

        def vaug_ap(np_, p, off, w):
            i, base = (0, 0) if p < 16 else (1, 16)
            return vaugh[i][:np_, (p - base) * 130 + off:(p - base) * 130 + off + w]

        def vtail_ap(np_, p, off, w):
            i, base = (0, 0) if p < 16 else (1, 16)
            return vtailh[i][:np_, (p - base) * 130 + off:(p - base) * 130 + off + w]

        # ---- chunked attention ----
        expp = ctx.enter_context(tc.tile_pool(name="expp", bufs=16))
        psS = ctx.enter_context(tc.tile_pool(name="psS", bufs=6, space="PSUM"))
        psOT = ctx.enter_context(tc.tile_pool(name="psOT", bufs=2, space="PSUM"))
        sot = ctx.enter_context(tc.tile_pool(name="sot", bufs=2))

        for c in (range(NCH) if chunks is None else chunks):
            # phase 1: scores + exp (+ masks) for BOTH heads, so the PE can
            # stream head 1's scores while head 0's exps drain on ACT.
            pieces_h = {}
            for h in heads:
                hq = slice(h * 64, (h + 1) * 64)
                hv = h * 65
                pieces = pieces_h[h] = []  # (et ap, vl ap, ot_col, w)

                # strided pairs: full-width pieces, one PSUM bank each; the
                # boundary-chunk prefix masking rides in the contraction
                # (indicator rows of ks x -1e9 rows of qs).
                qschunk = qsh[h][0 if c < 4 else 1][
                    :, (c % 4) * CHUNK:(c % 4 + 1) * CHUNK]
                for pr in range((c + 1) // 2):
                    npart = 128 if 2 * pr + 1 < c else 64
                    ps = psS.tile([128, CHUNK], f32, tag="psS")
                    nc.tensor.matmul(ps[:npart, :],
                                     ks[h][:, pr * 128:pr * 128 + npart],
                                     qschunk, start=True, stop=True)
                    et = expp.tile([128, CHUNK], bf16, tag="exp")
                    nc.scalar.activation(et[:npart, :], ps[:npart, :], Exp,
                                         scale=SM_SCALE)
                    pieces.append((et[:npart, :],
                                   vsaug[:npart, pr * 130 + hv:pr * 130 + hv + 65],
                                   0, CHUNK))

                # local: one piece per k-pair window (+ tail), packed into
                # shared PSUM banks so one exp serves several matmuls.
                locs = []  # (p, wlo, whi, npart, kind)
                for p in range(max(0, 4 * c - 4), min(31, 4 * c + 3) + 1):
                    if p == 4 * c - 4:
                        locs.append((p, 8 * c, 8 * c, 64, 2))     # tail
                        continue
                    wlo = max(8 * c, 2 * p)
                    whi = min(8 * c + 7, 2 * p + 8, 63)
                    if wlo > whi:
                        continue
                    locs.append((p, wlo, whi, 128, 0 if p >= 4 * c else 1))
                # first-fit-decreasing packing into 512-col banks
                banks = []  # [used, [(p, wlo, whi, npart, kind, off), ...]]
                for ent in sorted(locs, key=lambda e: -(e[2] - e[1] + 1)):
                    w = (ent[2] - ent[1] + 1) * 64
                    for bk in banks:
                        if bk[0] + w <= CHUNK:
                            bk[1].append(ent + (bk[0],))
                            bk[0] += w
                            break
                    else:
                        banks.append([w, [ent + (0,)]])
                for used, subs in banks:
                    ps = psS.tile([128, CHUNK], f32, tag="psS")
                    for p, wlo, whi, npart, kind, off in subs:
                        w = (whi - wlo + 1) * 64
                        if kind == 2:   # tail: single k-block 2p+1 vs i=8c
                            lhs = kT_ap(hq, (2 * p + 1) * 64, 64)
                        else:
                            lhs = kT_ap(hq, 2 * p * 64, 128)
                        nc.tensor.matmul(ps[:npart, off:off + w], lhs,
                                         qT_ap(hq, wlo * 64, w),
                                         start=True, stop=True,
                                         skip_group_check=True)
                    et = expp.tile([128, CHUNK], bf16, tag="exp")
                    nc.scalar.activation(et[:, :used], ps[:, :used], Exp,
                                         scale=SM_SCALE)
                    for p, wlo, whi, npart, kind, off in subs:
                        w = (whi - wlo + 1) * 64
                        if kind == 0:
                            nc.vector.tensor_mul(et[:, off:off + 128],
                                                 et[:, off:off + 128], m01[:])
                        elif kind == 1:
                            nc.vector.tensor_mul(et[:, off + w - 64:off + w],
                                                 et[:, off + w - 64:off + w],
                                                 mB[:])
                        if kind == 2:
                            vl = vtail_ap(npart, p, hv, 65)
                        else:
                            vl = vaug_ap(npart, p, hv, 65)
                        pieces.append((et[:npart, off:off + w], vl,
                                       (wlo - 8 * c) * 64, w))

            # phase 2: PV accumulation + output, per head. First piece must
            # cover the full 512 cols (start=True replaces a memset).
            for h in heads:
                pieces = pieces_h[h]
                ot = psOT.tile([65, CHUNK], f32, tag="psOT")
                ffull = next(i for i, pc in enumerate(pieces) if pc[3] == CHUNK)
                pieces[0], pieces[ffull] = pieces[ffull], pieces[0]
                for pi, (et, vl, col, w) in enumerate(pieces):
                    nc.tensor.matmul(ot[:, col:col + w], vl, et,
                                     start=(pi == 0), stop=(pi == len(pieces) - 1),
                                     skip_group_check=True)

                # [O^T; rowsums] to DRAM via SBUF staging; host normalizes.
                so = sot.tile([65, CHUNK], f32, tag="sot")
                nc.vector.tensor_copy(so[:], ot[:])
                nc.sync.dma_start(
                    outT_d[h * 65:(h + 1) * 65, c * CHUNK:(c + 1) * CHUNK],
                    so[:])

    return nc


def _in_maps(q, k, v):
    import ml_dtypes
    bf = ml_dtypes.bfloat16
    maps = []
    for c in range(NCORES):
        heads = [c, c + 8]
        s = (7 - c) % 8
        qT = np.ascontiguousarray(q[:, heads, :].reshape(SEQ, 128).T).astype(bf)
        kT = np.ascontiguousarray(k[:, heads, :].reshape(SEQ, 128).T).astype(bf)
        # strided contraction operands with boundary-bias augmentation:
        # ks[h] rows 64+b indicate packed block b's columns; qs[h] rows 64+b
        # carry -1e9 on chunk b+1's first s*64 columns.
        ksb = np.zeros((NSB * BS, 128), np.float32)
        vsb = np.zeros((NSB, BS, 128), np.float32)
        for b in range(7):
            j = s + 8 * b
            ksb[b * BS:(b + 1) * BS] = k[j * BS:(j + 1) * BS, heads, :].reshape(BS, 128)
            vsb[b] = v[j * BS:(j + 1) * BS, heads, :].reshape(BS, 128)
        ind = np.zeros((7, NSB * BS), np.float32)
        for b in range(7):
            ind[b, b * BS:(b + 1) * BS] = 1.0
        wrow = np.zeros((7, SEQ), np.float32)
        for b in range(7):
            wrow[b, (b + 1) * CHUNK:(b + 1) * CHUNK + s * 64] = -1e9
        qsl, ksl = [], []
        for hh in range(2):
            qs_h = np.concatenate(
                [q[:, heads[hh], :].T.astype(np.float32), wrow], axis=0)
            ks_h = np.concatenate(
                [ksb[:, hh * 64:(hh + 1) * 64].T, ind], axis=0)
            qsl.append(np.ascontiguousarray(qs_h).astype(bf))
            ksl.append(np.ascontiguousarray(ks_h).astype(bf))
        # vaug [128, 32*130]: pair a, token p -> [V_h0 | 1 | V_h1 | 1]
        vv = v[:, heads, :].reshape(32, 128, 128)   # [a, p, hd]
        vaug = np.ones((128, 32, 130), np.float32)
        vaug[:, :, 0:64] = vv.transpose(1, 0, 2)[:, :, 0:64]
        vaug[:, :, 65:129] = vv.transpose(1, 0, 2)[:, :, 64:128]
        # vsaug [128, 4*130]: pair pr: partitions 0-63 = block 2pr, 64-127 =
        # block 2pr+1
        vsp = vsb.reshape(4, 2, BS, 128).transpose(1, 2, 0, 3).reshape(128, 4, 128)
        vsaug = np.ones((128, 4, 130), np.float32)
        vsaug[:, :, 0:64] = vsp[:, :, 0:64]
        vsaug[:, :, 65:129] = vsp[:, :, 64:128]
        # vtail [64, 32*130]: odd blocks 2a+1
        vt = v[:, heads, :].reshape(32, 2, 64, 128)[:, 1]   # [a, p, hd]
        vtail = np.ones((64, 32, 130), np.float32)
        vtail[:, :, 0:64] = vt.transpose(1, 0, 2)[:, :, 0:64]
        vtail[:, :, 65:129] = vt.transpose(1, 0, 2)[:, :, 64:128]
        maps.append({"qT": qT, "kT": kT,
                     "qs0": qsl[0], "qs1": qsl[1],
                     "ks0": ksl[0], "ks1": ksl[1],
                     "vaug": vaug.reshape(128, 32 * 130).astype(bf),
                     "vsaug": vsaug.reshape(128, 4 * 130).astype(bf),
                     "vtail": vtail.reshape(64, 32 * 130).astype(bf)})
    return maps


def kernel(q, k, v, cu_seqlens_k=None, **_):
    from concourse.bass_utils import run_bass_kernel_spmd

    q = np.asarray(q, np.float32)
    k = np.asarray(k, np.float32)
    v = np.asarray(v, np.float32)
    if "nc" not in _cache:
        _cache["nc"] = _legalize_waits(_build_program())
    res = run_bass_kernel_spmd(_cache["nc"], _in_maps(q, k, v),
                               list(range(NCORES))).results
    out = np.empty((SEQ, N_HEADS, HEAD), np.float32)
    for c in range(NCORES):
        o = res[c]["outT"]                      # [130, SEQ]
        for hh, head in ((0, c), (1, c + 8)):
            num = o[hh * 65:hh * 65 + 64, :]    # [64, SEQ]
            den = o[hh * 65 + 64, :]            # [SEQ]
            out[:, head, :] = (num / den).T
    return out


# revision 15
# speedup vs baseline: 3.1204x; 1.0905x over previous
"""Block-sparse local+strided attention (LocalStridedBlockSparseAttn) on 8 trn2 cores.

Problem: q,k,v [4096, 16, 64] f32, single prefill sequence. Per-head block mask
(64x64 token blocks): j <= i and (i - j < 8  or  (j + h + 1) % 8 == 0).

Sharding: core c owns heads {c, c+8} - both have the same strided residue
s = (7 - c) % 8, so one SPMD program serves all 8 cores with per-core data.

v4 (instruction-minimal dataflow; v3 was PE-bound on per-instruction fixed
costs and startup DMA serialization):
  - local part per (chunk, head): one [128, w] matmul per k-block PAIR over
    its contiguous valid q-window (w up to 512), masked post-exp with small
    constant masks, plus one 64x64 tail piece; small pieces packed into
    shared PSUM banks so ONE activation serves several matmuls.
  - strided validity boundary folded into the CONTRACTION: ks carries 7
    indicator partitions and the strided q copy carries -1e9 rows on the
    boundary-chunk prefix columns (zero per-piece instructions).
  - output stays TRANSPOSED with the rowsums row: the [65, 512] PSUM tile
    [O^T; rowsums] is DMA'd straight to DRAM; the host does the divide and
    the final transpose. No PE transposes, no reciprocal/normalize/copy
    instructions on device.
  - big inputs split in halves with chunk-0-critical slices DMA'd first so
    compute starts before the tail of the input load.
All matmul operands bf16; exp outputs bf16 (PSUM accumulates fp32).
"""

import numpy as np

N_HEADS = 16
HEAD = 64
SEQ = 4096
BS = 64
NB = 64          # 64 token-blocks
LOCAL = 8
VERT = 8
SM_SCALE = 1.0 / 8.0
NCORES = 8
CHUNK = 512      # q tokens per chunk (8 blocks)
NCH = SEQ // CHUNK
NSB = 8          # packed strided block slots (7 real, slot 7 zero pad)
KSP = 64 + 7     # ks/qs partitions: 64 head dims + 7 boundary indicator rows
HSEQ = SEQ // 2

_cache = {}


def _legalize_waits(nc, max_waits=1):
    """This walrus build rejects instructions carrying more than one sync-wait
    condition ("Too many sync wait commands"); hoist extras into same-engine
    NoOps placed immediately before the instruction."""
    import concourse.mybir as mybir

    nid = 0
    for bb in nc.main_func.blocks:
        new = []
        for ins in bb.instructions:
            si = ins.sync_info
            if si is not None and si.on_wait and len(si.on_wait) > max_waits:
                waits = list(si.on_wait)
                while len(waits) > max_waits:
                    chunk, waits = waits[:max_waits], waits[max_waits:]
                    nid += 1
                    nop = mybir.InstNoOp(name=f"{ins.name}-wsplit{nid}")
                    nop.engine = ins.engine
                    nop.sync_info = mybir.SyncInfo(on_wait=chunk, on_update=[])
                    new.append(nop)
                ins.sync_info = mybir.SyncInfo(on_wait=waits,
                                               on_update=list(si.on_update))
            new.append(ins)
        bb.instructions[:] = new
    return nc


def _build_program(chunks=None, heads=(0, 1)):
    from contextlib import ExitStack

    import concourse.bass as bass
    import concourse.mybir as mybir
    from concourse import tile

    f32 = mybir.dt.float32
    bf16 = mybir.dt.bfloat16
    Exp = mybir.ActivationFunctionType.Exp

    nc = bass.Bass()
    qT_d = nc.dram_tensor("qT", [128, SEQ], bf16, kind="ExternalInput")
    kT_d = nc.dram_tensor("kT", [128, SEQ], bf16, kind="ExternalInput")
    qs_d = [nc.dram_tensor(f"qs{h}", [KSP, SEQ], bf16, kind="ExternalInput")
            for h in range(2)]
    ks_d = [nc.dram_tensor(f"ks{h}", [KSP, NSB * BS], bf16, kind="ExternalInput")
            for h in range(2)]
    vaug_d = nc.dram_tensor("vaug", [128, 32 * 130], bf16, kind="ExternalInput")
    vsaug_d = nc.dram_tensor("vsaug", [128, 4 * 130], bf16, kind="ExternalInput")
    vtail_d = nc.dram_tensor("vtail", [64, 32 * 130], bf16, kind="ExternalInput")
    # transposed output with rowsums: rows h*65..h*65+63 = O^T, row h*65+64 =
    # softmax denominators; host divides + transposes back.
    outT_d = nc.dram_tensor("outT", [130, SEQ], f32, kind="ExternalOutput")

    # Device-constant tiles (same on every core).
    import ml_dtypes
    bf = ml_dtypes.bfloat16
    n = np.arange(64)
    tri = (n[None, :] >= n[:, None]).astype(np.float32)
    m01_np = np.zeros((128, 128), np.float32)
    m01_np[:64, :64] = tri          # q-block 2p vs k-block 2p
    m01_np[:64, 64:] = 1.0          # q-block 2p+1 vs k-block 2p
    m01_np[64:, 64:] = tri          # q-block 2p+1 vs k-block 2p+1
    mB_np = np.zeros((128, 64), np.float32)
    mB_np[64:] = 1.0                # q-block 2p+8: only k-block 2p+1 valid
    m01_d = nc.inline_tensor(m01_np.astype(bf), "m01_c")
    mB_d = nc.inline_tensor(mB_np.astype(bf), "mB_c")

    with tile.TileContext(nc) as tc, ExitStack() as ctx:
        const = ctx.enter_context(tc.tile_pool(name="const", bufs=1))
        m01 = const.tile([128, 128], bf16, tag="m01")
        mB = const.tile([128, 64], bf16, tag="mB")
        nc.sync.dma_start(m01[:], m01_d[:])
        nc.sync.dma_start(mB[:], mB_d[:])

        big = ctx.enter_context(tc.tile_pool(name="big", bufs=1))
        # quarters (1024 cols) so chunk 0 starts after ~1/4 of the load;
        # issue on BOTH hwdge queues (sync + scalar) in need-order.
        kTq = [big.tile([128, 1024], bf16, tag=f"kT{i}", name=f"kT{i}")
               for i in range(4)]
        qTq = [big.tile([128, 1024], bf16, tag=f"qT{i}", name=f"qT{i}")
               for i in range(4)]
        qsh = [[big.tile([KSP, HSEQ], bf16, tag=f"qs{h}_{i}", name=f"qs{h}_{i}")
                for i in range(2)] for h in range(2)]
        ks = [big.tile([KSP, NSB * BS], bf16, tag=f"ks{h}", name=f"ks{h}")
              for h in range(2)]
        vaugh = [big.tile([128, 16 * 130], bf16, tag=f"vaug{i}", name=f"vaug{i}")
                 for i in range(2)]
        vsaug = big.tile([128, 4 * 130], bf16, tag="vsaug")
        vtailh = [big.tile([64, 16 * 130], bf16, tag=f"vtail{i}", name=f"vtail{i}")
                  for i in range(2)]
        # scalar queue: ONLY small early tensors (the scalar sequencer must
        # drain its DMA issues before it can run the first exp — keep it light)
        for h in range(2):
            nc.scalar.dma_start(ks[h][:], ks_d[h][:])
        nc.scalar.dma_start(vsaug[:], vsaug_d[:])
        # sync queue: everything else in need-order
        nc.sync.dma_start(kTq[0][:], kT_d[:, 0:1024])
        nc.sync.dma_start(qTq[0][:], qT_d[:, 0:1024])
        nc.sync.dma_start(vaugh[0][:], vaug_d[:, 0:16 * 130])
        nc.sync.dma_start(vtailh[0][:], vtail_d[:, 0:16 * 130])
        nc.sync.dma_start(kTq[1][:], kT_d[:, 1024:2048])
        nc.sync.dma_start(qTq[1][:], qT_d[:, 1024:2048])
        for h in range(2):
            nc.sync.dma_start(qsh[h][0][:], qs_d[h][:, 0:HSEQ])
        nc.sync.dma_start(kTq[2][:], kT_d[:, 2048:3072])
        nc.sync.dma_start(qTq[2][:], qT_d[:, 2048:3072])
        nc.sync.dma_start(vaugh[1][:], vaug_d[:, 16 * 130:32 * 130])
        nc.sync.dma_start(vtailh[1][:], vtail_d[:, 16 * 130:32 * 130])
        nc.sync.dma_start(kTq[3][:], kT_d[:, 3072:SEQ])
        nc.sync.dma_start(qTq[3][:], qT_d[:, 3072:SEQ])
        for h in range(2):
            nc.sync.dma_start(qsh[h][1][:], qs_d[h][:, HSEQ:SEQ])

        def kT_ap(hq, col, w):
            return kTq[col // 1024][hq, col % 1024:col % 1024 + w]

        def qT_ap(hq, col, w):
            return qTq[col // 1024][hq, col % 1024:col % 1024 + w]

        def vaug_ap(np_, p, off, w):
            i, base = (0, 0) if p < 16 else (1, 16)
            return vaugh[i][:np_, (p - base) * 130 + off:(p - base) * 130 + off + w]

        def vtail_ap(np_, p, off, w):
            i, base = (0, 0) if p < 16 else (1, 16)
            return vtailh[i][:np_, (p - base) * 130 + off:(p - base) * 130 + off + w]

        # ---- chunked attention ----
        expp = ctx.enter_context(tc.tile_pool(name="expp", bufs=16))
        psS = ctx.enter_context(tc.tile_pool(name="psS", bufs=6, space="PSUM"))
        psOT = ctx.enter_context(tc.tile_pool(name="psOT", bufs=2, space="PSUM"))
        sot = ctx.enter_context(tc.tile_pool(name="sot", bufs=2))

        for c in (range(NCH) if chunks is None else chunks):
            # phase 1: scores + exp (+ masks) for BOTH heads, so the PE can
            # stream head 1's scores while head 0's exps drain on ACT.
            pieces_h = {}
            for h in heads:
                hq = slice(h * 64, (h + 1) * 64)
                hv = h * 65
                pieces = pieces_h[h] = []  # (et ap, vl ap, ot_col, w)

                # strided pairs: full-width pieces, one PSUM bank each; the
                # boundary-chunk prefix masking rides in the contraction
                # (indicator rows of ks x -1e9 rows of qs).
                qschunk = qsh[h][0 if c < 4 else 1][
                    :, (c % 4) * CHUNK:(c % 4 + 1) * CHUNK]
                for pr in range((c + 1) // 2):
                    npart = 128 if 2 * pr + 1 < c else 64
                    ps = psS.tile([128, CHUNK], f32, tag="psS")
                    nc.tensor.matmul(ps[:npart, :],
                                     ks[h][:, pr * 128:pr * 128 + npart],
                                     qschunk, start=True, stop=True)
                    et = expp.tile([128, CHUNK], bf16, tag="exp")
                    nc.scalar.activation(et[:npart, :], ps[:npart, :], Exp,
                                         scale=SM_SCALE)
                    pieces.append((et[:npart, :],
                                   vsaug[:npart, pr * 130 + hv:pr * 130 + hv + 65],
                                   0, CHUNK))

                # local: one piece per k-pair window (+ tail), packed into
                # shared PSUM banks so one exp serves several matmuls.
                locs = []  # (p, wlo, whi, npart, kind)
                for p in range(max(0, 4 * c - 4), min(31, 4 * c + 3) + 1):
                    if p == 4 * c - 4:
                        locs.append((p, 8 * c, 8 * c, 64, 2))     # tail
                        continue
                    wlo = max(8 * c, 2 * p)
                    whi = min(8 * c + 7, 2 * p + 8, 63)
                    if wlo > whi:
                        continue
                    locs.append((p, wlo, whi, 128, 0 if p >= 4 * c else 1))
                # first-fit-decreasing packing into 512-col banks
                banks = []  # [used, [(p, wlo, whi, npart, kind, off), ...]]
                for ent in sorted(locs, key=lambda e: -(e[2] - e[1] + 1)):
                    w = (ent[2] - ent[1] + 1) * 64
                    for bk in banks:
                        if bk[0] + w <= CHUNK:
                            bk[1].append(ent + (bk[0],))
                            bk[0] += w
                            break
                    else:
                        banks.append([w, [ent + (0,)]])
                for used, subs in banks:
                    ps = psS.tile([128, CHUNK], f32, tag="psS")
                    for p, wlo, whi, npart, kind, off in subs:
                        w = (whi - wlo + 1) * 64
                        if kind == 2:   # tail: single k-block 2p+1 vs i=8c
                            lhs = kT_ap(hq, (2 * p + 1) * 64, 64)
                        else:
                            lhs = kT_ap(hq, 2 * p * 64, 128)
                        nc.tensor.matmul(ps[:npart, off:off + w], lhs,
                                         qT_ap(hq, wlo * 64, w),
                                         start=True, stop=True,
                                         skip_group_check=True)
                    et = expp.tile([128, CHUNK], bf16, tag="exp")
                    nc.scalar.activation(et[:, :used], ps[:, :used], Exp,
                                         scale=SM_SCALE)
                    for p, wlo, whi, npart, kind, off in subs:
                        w = (whi - wlo + 1) * 64
                        if kind == 0:
                            nc.vector.tensor_mul(et[:, off:off + 128],
                                                 et[:, off:off + 128], m01[:])
                        elif kind == 1:
                            nc.vector.tensor_mul(et[:, off + w - 64:off + w],
                                                 et[:, off + w - 64:off + w],
                                                 mB[:])
                        if kind == 2:
                            vl = vtail_ap(npart, p, hv, 65)
                        else:
                            vl = vaug_ap(npart, p, hv, 65)
                        pieces.append((et[:npart, off:off + w], vl,
                                       (wlo - 8 * c) * 64, w))

            # phase 2: PV accumulation + output, per head. First piece must
            # cover the full 512 cols (start=True replaces a memset).
            for h in heads:
                pieces = pieces_h[h]
                ot = psOT.tile([65, CHUNK], f32, tag="psOT")
                ffull = next(i for i, pc in enumerate(pieces) if pc[3] == CHUNK)
                pieces[0], pieces[ffull] = pieces[ffull], pieces[0]
                for pi, (et, vl, col, w) in enumerate(pieces):
                    nc.tensor.matmul(ot[:, col:col + w], vl, et,
                                     start=(pi == 0), stop=(pi == len(pieces) - 1),
                                     skip_group_check=True)

                # [O^T; rowsums] to DRAM via SBUF staging; host normalizes.
                so = sot.tile([65, CHUNK], f32, tag="sot")
                nc.vector.tensor_copy(so[:], ot[:])
                nc.sync.dma_start(
                    outT_d[h * 65:(h + 1) * 65, c * CHUNK:(c + 1) * CHUNK],
                    so[:])

    return nc


def _in_maps(q, k, v):
    import ml_dtypes
    bf = ml_dtypes.bfloat16
    maps = []
    for c in range(NCORES):
        heads = [c, c + 8]
        s = (7 - c) % 8
        qT = np.ascontiguousarray(q[:, heads, :].reshape(SEQ, 128).T).astype(bf)
        kT = np.ascontiguousarray(k[:, heads, :].reshape(SEQ, 128).T).astype(bf)
        # strided contraction operands with boundary-bias augmentation:
        # ks[h] rows 64+b indicate packed block b's columns; qs[h] rows 64+b
        # carry -1e9 on chunk b+1's first s*64 columns.
        ksb = np.zeros((NSB * BS, 128), np.float32)
        vsb = np.zeros((NSB, BS, 128), np.float32)
        for b in range(7):
            j = s + 8 * b
            ksb[b * BS:(b + 1) * BS] = k[j * BS:(j + 1) * BS, heads, :].reshape(BS, 128)
            vsb[b] = v[j * BS:(j + 1) * BS, heads, :].reshape(BS, 128)
        ind = np.zeros((7, NSB * BS), np.float32)
        for b in range(7):
            ind[b, b * BS:(b + 1) * BS] = 1.0
        wrow = np.zeros((7, SEQ), np.float32)
        for b in range(7):
            wrow[b, (b + 1) * CHUNK:(b + 1) * CHUNK + s * 64] = -1e9
        qsl, ksl = [], []
        for hh in range(2):
            qs_h = np.concatenate(
                [q[:, heads[hh], :].T.astype(np.float32), wrow], axis=0)
            ks_h = np.concatenate(
                [ksb[:, hh * 64:(hh + 1) * 64].T, ind], axis=0)
            qsl.append(np.ascontiguousarray(qs_h).astype(bf))
            ksl.append(np.ascontiguousarray(ks_h).astype(bf))
        # vaug [128, 32*130]: pair a, token p -> [V_h0 | 1 | V_h1 | 1]
        vv = v[:, heads, :].reshape(32, 128, 128)   # [a, p, hd]
        vaug = np.ones((128, 32, 130), np.float32)
        vaug[:, :, 0:64] = vv.transpose(1, 0, 2)[:, :, 0:64]
        vaug[:, :, 65:129] = vv.transpose(1, 0, 2)[:, :, 64:128]
        # vsaug [128, 4*130]: pair pr: partitions 0-63 = block 2pr, 64-127 =
        # block 2pr+1
        vsp = vsb.reshape(4, 2, BS, 128).transpose(1, 2, 0, 3).reshape(128, 4, 128)
        vsaug = np.ones((128, 4, 130), np.float32)
        vsaug[:, :, 0:64] = vsp[:, :, 0:64]
        vsaug[:, :, 65:129] = vsp[:, :, 64:128]
        # vtail [64, 32*130]: odd blocks 2a+1
        vt = v[:, heads, :].reshape(32, 2, 64, 128)[:, 1]   # [a, p, hd]
        vtail = np.ones((64, 32, 130), np.float32)
        vtail[:, :, 0:64] = vt.transpose(1, 0, 2)[:, :, 0:64]
        vtail[:, :, 65:129] = vt.transpose(1, 0, 2)[:, :, 64:128]
        maps.append({"qT": qT, "kT": kT,
                     "qs0": qsl[0], "qs1": qsl[1],
                     "ks0": ksl[0], "ks1": ksl[1],
                     "vaug": vaug.reshape(128, 32 * 130).astype(bf),
                     "vsaug": vsaug.reshape(128, 4 * 130).astype(bf),
                     "vtail": vtail.reshape(64, 32 * 130).astype(bf)})
    return maps


def kernel(q, k, v, cu_seqlens_k=None, **_):
    from concourse.bass_utils import run_bass_kernel_spmd

    q = np.asarray(q, np.float32)
    k = np.asarray(k, np.float32)
    v = np.asarray(v, np.float32)
    if "nc" not in _cache:
        _cache["nc"] = _legalize_waits(_build_program())
    res = run_bass_kernel_spmd(_cache["nc"], _in_maps(q, k, v),
                               list(range(NCORES))).results
    out = np.empty((SEQ, N_HEADS, HEAD), np.float32)
    for c in range(NCORES):
        o = res[c]["outT"]                      # [130, SEQ]
        for hh, head in ((0, c), (1, c + 8)):
            num = o[hh * 65:hh * 65 + 64, :]    # [64, SEQ]
            den = o[hh * 65 + 64, :]            # [SEQ]
            out[:, head, :] = (num / den).T
    return out


# revision 18
# speedup vs baseline: 3.6339x; 1.1645x over previous
"""Block-sparse local+strided attention (LocalStridedBlockSparseAttn) on 8 trn2 cores.

Problem: q,k,v [4096, 16, 64] f32, single prefill sequence. Per-head block mask
(64x64 token blocks): j <= i and (i - j < 8  or  (j + h + 1) % 8 == 0).

Sharding: core c owns heads {c, c+8} - both have the same strided residue
s = (7 - c) % 8, so one SPMD program serves all 8 cores with per-core data.

v4 (instruction-minimal dataflow; v3 was PE-bound on per-instruction fixed
costs and startup DMA serialization):
  - local part per (chunk, head): one [128, w] matmul per k-block PAIR over
    its contiguous valid q-window (w up to 512), masked post-exp with small
    constant masks, plus one 64x64 tail piece; small pieces packed into
    shared PSUM banks so ONE activation serves several matmuls.
  - strided validity boundary folded into the CONTRACTION: ks carries 7
    indicator partitions and the strided q copy carries -1e9 rows on the
    boundary-chunk prefix columns (zero per-piece instructions).
  - output stays TRANSPOSED with the rowsums row: the [65, 512] PSUM tile
    [O^T; rowsums] is DMA'd straight to DRAM; the host does the divide and
    the final transpose. No PE transposes, no reciprocal/normalize/copy
    instructions on device.
  - big inputs split in halves with chunk-0-critical slices DMA'd first so
    compute starts before the tail of the input load.
All matmul operands bf16; exp outputs bf16 (PSUM accumulates fp32).
"""

import numpy as np

N_HEADS = 16
HEAD = 64
SEQ = 4096
BS = 64
NB = 64          # 64 token-blocks
LOCAL = 8
VERT = 8
SM_SCALE = 1.0 / 8.0
NCORES = 8
CHUNK = 512      # q tokens per chunk (8 blocks)
NCH = SEQ // CHUNK
NSB = 8          # packed strided block slots (7 real, slot 7 zero pad)
KSP = 64 + 7     # ks/qs partitions: 64 head dims + 7 boundary indicator rows
HSEQ = SEQ // 2

_cache = {}


def _legalize_waits(nc, max_waits=1):
    """This walrus build rejects instructions carrying more than one sync-wait
    condition ("Too many sync wait commands"); hoist extras into same-engine
    NoOps placed immediately before the instruction."""
    import concourse.mybir as mybir

    nid = 0
    for bb in nc.main_func.blocks:
        new = []
        for ins in bb.instructions:
            si = ins.sync_info
            if si is not None and si.on_wait and len(si.on_wait) > max_waits:
                waits = list(si.on_wait)
                while len(waits) > max_waits:
                    chunk, waits = waits[:max_waits], waits[max_waits:]
                    nid += 1
                    nop = mybir.InstNoOp(name=f"{ins.name}-wsplit{nid}")
                    nop.engine = ins.engine
                    nop.sync_info = mybir.SyncInfo(on_wait=chunk, on_update=[])
                    new.append(nop)
                ins.sync_info = mybir.SyncInfo(on_wait=waits,
                                               on_update=list(si.on_update))
            new.append(ins)
        bb.instructions[:] = new
    return nc


def _build_program(chunks=None, heads=(0, 1)):
    from contextlib import ExitStack

    import concourse.bass as bass
    import concourse.mybir as mybir
    from concourse import tile

    f32 = mybir.dt.float32
    bf16 = mybir.dt.bfloat16
    Exp = mybir.ActivationFunctionType.Exp

    nc = bass.Bass()
    qT_d = nc.dram_tensor("qT", [128, SEQ], bf16, kind="ExternalInput")
    kT_d = nc.dram_tensor("kT", [128, SEQ], bf16, kind="ExternalInput")
    qs_d = [nc.dram_tensor(f"qs{h}", [KSP, SEQ], bf16, kind="ExternalInput")
            for h in range(2)]
    ks_d = [nc.dram_tensor(f"ks{h}", [KSP, NSB * BS], bf16, kind="ExternalInput")
            for h in range(2)]
    vaug_d = nc.dram_tensor("vaug", [128, 32 * 130], bf16, kind="ExternalInput")
    vsaug_d = nc.dram_tensor("vsaug", [128, 4 * 130], bf16, kind="ExternalInput")
    vtail_d = nc.dram_tensor("vtail", [64, 32 * 130], bf16, kind="ExternalInput")
    # transposed output with rowsums: rows h*65..h*65+63 = O^T, row h*65+64 =
    # softmax denominators; host divides + transposes back.
    outT_d = nc.dram_tensor("outT", [130, SEQ], f32, kind="ExternalOutput")

    # Device-constant tiles (same on every core).
    import ml_dtypes
    bf = ml_dtypes.bfloat16
    n = np.arange(64)
    tri = (n[None, :] >= n[:, None]).astype(np.float32)
    m01_np = np.zeros((128, 128), np.float32)
    m01_np[:64, :64] = tri          # q-block 2p vs k-block 2p
    m01_np[:64, 64:] = 1.0          # q-block 2p+1 vs k-block 2p
    m01_np[64:, 64:] = tri          # q-block 2p+1 vs k-block 2p+1
    mB_np = np.zeros((128, 64), np.float32)
    mB_np[64:] = 1.0                # q-block 2p+8: only k-block 2p+1 valid
    m01_d = nc.inline_tensor(m01_np.astype(bf), "m01_c")
    mB_d = nc.inline_tensor(mB_np.astype(bf), "mB_c")

    with tile.TileContext(nc) as tc, ExitStack() as ctx:
        const = ctx.enter_context(tc.tile_pool(name="const", bufs=1))
        m01 = const.tile([128, 128], bf16, tag="m01")
        mB = const.tile([128, 64], bf16, tag="mB")
        nc.sync.dma_start(m01[:], m01_d[:])
        nc.sync.dma_start(mB[:], mB_d[:])

        big = ctx.enter_context(tc.tile_pool(name="big", bufs=1))
        # quarters (1024 cols) so chunk 0 starts after ~1/4 of the load;
        # issue on BOTH hwdge queues (sync + scalar) in need-order.
        kTq = [big.tile([128, 1024], bf16, tag=f"kT{i}", name=f"kT{i}")
               for i in range(4)]
        qTq = [big.tile([128, 1024], bf16, tag=f"qT{i}", name=f"qT{i}")
               for i in range(4)]
        qsh = [[big.tile([KSP, HSEQ], bf16, tag=f"qs{h}_{i}", name=f"qs{h}_{i}")
                for i in range(2)] for h in range(2)]
        ks = [big.tile([KSP, NSB * BS], bf16, tag=f"ks{h}", name=f"ks{h}")
              for h in range(2)]
        vaugh = [big.tile([128, 16 * 130], bf16, tag=f"vaug{i}", name=f"vaug{i}")
                 for i in range(2)]
        vsaug = big.tile([128, 4 * 130], bf16, tag="vsaug")
        vtailh = [big.tile([64, 16 * 130], bf16, tag=f"vtail{i}", name=f"vtail{i}")
                  for i in range(2)]
        # scalar queue: small early tensors + chunk-0/1 V layouts (the scalar
        # sequencer must drain its DMA issues before it can run the first exp)
        for h in range(2):
            nc.scalar.dma_start(ks[h][:], ks_d[h][:])
        nc.scalar.dma_start(vsaug[:], vsaug_d[:])
        nc.scalar.dma_start(vaugh[0][:], vaug_d[:, 0:16 * 130])
        nc.scalar.dma_start(vtailh[0][:], vtail_d[:, 0:16 * 130])
        # sync queue: everything else in need-order
        nc.sync.dma_start(kTq[0][:], kT_d[:, 0:1024])
        nc.sync.dma_start(qTq[0][:], qT_d[:, 0:1024])
        for h in range(2):
            nc.sync.dma_start(qsh[h][0][:], qs_d[h][:, 0:HSEQ])
        nc.sync.dma_start(kTq[1][:], kT_d[:, 1024:2048])
        nc.sync.dma_start(qTq[1][:], qT_d[:, 1024:2048])
        nc.sync.dma_start(kTq[2][:], kT_d[:, 2048:3072])
        nc.sync.dma_start(qTq[2][:], qT_d[:, 2048:3072])
        nc.sync.dma_start(vaugh[1][:], vaug_d[:, 16 * 130:32 * 130])
        nc.sync.dma_start(vtailh[1][:], vtail_d[:, 16 * 130:32 * 130])
        nc.sync.dma_start(kTq[3][:], kT_d[:, 3072:SEQ])
        nc.sync.dma_start(qTq[3][:], qT_d[:, 3072:SEQ])
        for h in range(2):
            nc.sync.dma_start(qsh[h][1][:], qs_d[h][:, HSEQ:SEQ])

        def kT_ap(hq, col, w):
            return kTq[col // 1024][hq, col % 1024:col % 1024 + w]

        def qT_ap(hq, col, w):
            return qTq[col // 1024][hq, col % 1024:col % 1024 + w]

        def vaug_ap(np_, p, off, w):
            i, base = (0, 0) if p < 16 else (1, 16)
            return vaugh[i][:np_, (p - base) * 130 + off:(p - base) * 130 + off + w]

        def vtail_ap(np_, p, off, w):
            i, base = (0, 0) if p < 16 else (1, 16)
            return vtailh[i][:np_, (p - base) * 130 + off:(p - base) * 130 + off + w]

        # ---- chunked attention ----
        # psD tiles span TWO PSUM banks ([128, 1024]) so one exp instruction
        # serves two 512-col score groups.
        expp = ctx.enter_context(tc.tile_pool(name="expp", bufs=10))
        psD = ctx.enter_context(tc.tile_pool(name="psD", bufs=3, space="PSUM"))
        psOT = ctx.enter_context(tc.tile_pool(name="psOT", bufs=2, space="PSUM"))
        sot = ctx.enter_context(tc.tile_pool(name="sot", bufs=2))

        for c in (range(NCH) if chunks is None else chunks):
            # phase 1: scores + exp (+ masks) for BOTH heads, so the PE can
            # stream head 1's scores while head 0's exps drain on ACT.
            pieces_h = {}
            for h in heads:
                hq = slice(h * 64, (h + 1) * 64)
                hv = h * 65
                pieces = pieces_h[h] = []  # (et ap, vl ap, ot_col, w)

                # build 512-col sub-bank groups first, then pair them into
                # two-bank [128, 1024] PSUM tiles with ONE exp each.
                # strided pieces (full-width; boundary masking rides in the
                # contraction via ks indicator rows x qs -1e9 rows):
                qschunk = qsh[h][0 if c < 4 else 1][
                    :, (c % 4) * CHUNK:(c % 4 + 1) * CHUNK]
                subbanks = []  # (used, [(p, wlo, whi, npart, kind, off)])
                for pr in range((c + 1) // 2):
                    npart = 128 if 2 * pr + 1 < c else 64
                    subbanks.append((CHUNK, [(pr, 8 * c, 8 * c + 7, npart, 3, 0)]))
                # local: one piece per k-pair window (+ tail)
                locs = []  # (p, wlo, whi, npart, kind)
                for p in range(max(0, 4 * c - 4), min(31, 4 * c + 3) + 1):
                    if p == 4 * c - 4:
                        locs.append((p, 8 * c, 8 * c, 64, 2))     # tail
                        continue
                    wlo = max(8 * c, 2 * p)
                    whi = min(8 * c + 7, 2 * p + 8, 63)
                    if wlo > whi:
                        continue
                    locs.append((p, wlo, whi, 128, 0 if p >= 4 * c else 1))
                banks = []  # [used, [(p, wlo, whi, npart, kind, off), ...]]
                for ent in sorted(locs, key=lambda e: -(e[2] - e[1] + 1)):
                    w = (ent[2] - ent[1] + 1) * 64
                    for bk in banks:
                        if bk[0] + w <= CHUNK:
                            bk[1].append(ent + (bk[0],))
                            bk[0] += w
                            break
                    else:
                        banks.append([w, [ent + (0,)]])
                subbanks.extend((u, s) for u, s in banks)

                for g in range(0, len(subbanks), 2):
                    pair = subbanks[g:g + 2]
                    ps = psD.tile([128, 2 * CHUNK], f32, tag="psD")
                    et = expp.tile([128, 2 * CHUNK], bf16, tag="exp")
                    for half, (used, subs) in enumerate(pair):
                        hb = half * CHUNK
                        for p, wlo, whi, npart, kind, off in subs:
                            w = (whi - wlo + 1) * 64
                            if kind == 3:
                                nc.tensor.matmul(
                                    ps[:npart, hb:hb + CHUNK],
                                    ks[h][:, p * 128:p * 128 + npart],
                                    qschunk, start=True, stop=True,
                                    skip_group_check=True)
                            else:
                                lhs = (kT_ap(hq, (2 * p + 1) * 64, 64)
                                       if kind == 2 else
                                       kT_ap(hq, 2 * p * 64, 128))
                                nc.tensor.matmul(
                                    ps[:npart, hb + off:hb + off + w], lhs,
                                    qT_ap(hq, wlo * 64, w),
                                    start=True, stop=True,
                                    skip_group_check=True)
                    width = (CHUNK + pair[1][0]) if len(pair) == 2 else pair[0][0]
                    nc.scalar.activation(et[:, :width], ps[:, :width], Exp,
                                         scale=SM_SCALE)
                    for half, (used, subs) in enumerate(pair):
                        hb = half * CHUNK
                        for p, wlo, whi, npart, kind, off in subs:
                            w = (whi - wlo + 1) * 64
                            if kind == 0:
                                nc.vector.tensor_mul(
                                    et[:, hb + off:hb + off + 128],
                                    et[:, hb + off:hb + off + 128], m01[:])
                            elif kind == 1:
                                nc.vector.tensor_mul(
                                    et[:, hb + off + w - 64:hb + off + w],
                                    et[:, hb + off + w - 64:hb + off + w],
                                    mB[:])
                            if kind == 3:
                                vl = vsaug[:npart, p * 130 + hv:p * 130 + hv + 65]
                            elif kind == 2:
                                vl = vtail_ap(npart, p, hv, 65)
                            else:
                                vl = vaug_ap(npart, p, hv, 65)
                            pieces.append((et[:npart, hb + off:hb + off + w], vl,
                                           (wlo - 8 * c) * 64, w))

            # phase 2: PV accumulation + output, per head. First piece must
            # cover the full 512 cols (start=True replaces a memset).
            for h in heads:
                pieces = pieces_h[h]
                ot = psOT.tile([65, CHUNK], f32, tag="psOT")
                ffull = next(i for i, pc in enumerate(pieces) if pc[3] == CHUNK)
                pieces[0], pieces[ffull] = pieces[ffull], pieces[0]
                for pi, (et, vl, col, w) in enumerate(pieces):
                    nc.tensor.matmul(ot[:, col:col + w], vl, et,
                                     start=(pi == 0), stop=(pi == len(pieces) - 1),
                                     skip_group_check=True)

                # [O^T; rowsums] to DRAM via SBUF staging; host normalizes.
                so = sot.tile([65, CHUNK], f32, tag="sot")
                nc.vector.tensor_copy(so[:], ot[:])
                nc.sync.dma_start(
                    outT_d[h * 65:(h + 1) * 65, c * CHUNK:(c + 1) * CHUNK],
                    so[:])

    return nc


def _in_maps(q, k, v):
    import ml_dtypes
    bf = ml_dtypes.bfloat16
    maps = []
    for c in range(NCORES):
        heads = [c, c + 8]
        s = (7 - c) % 8
        qT = np.ascontiguousarray(q[:, heads, :].reshape(SEQ, 128).T).astype(bf)
        kT = np.ascontiguousarray(k[:, heads, :].reshape(SEQ, 128).T).astype(bf)
        # strided contraction operands with boundary-bias augmentation:
        # ks[h] rows 64+b indicate packed block b's columns; qs[h] rows 64+b
        # carry -1e9 on chunk b+1's first s*64 columns.
        ksb = np.zeros((NSB * BS, 128), np.float32)
        vsb = np.zeros((NSB, BS, 128), np.float32)
        for b in range(7):
            j = s + 8 * b
            ksb[b * BS:(b + 1) * BS] = k[j * BS:(j + 1) * BS, heads, :].reshape(BS, 128)
            vsb[b] = v[j * BS:(j + 1) * BS, heads, :].reshape(BS, 128)
        ind = np.zeros((7, NSB * BS), np.float32)
        for b in range(7):
            ind[b, b * BS:(b + 1) * BS] = 1.0
        wrow = np.zeros((7, SEQ), np.float32)
        for b in range(7):
            wrow[b, (b + 1) * CHUNK:(b + 1) * CHUNK + s * 64] = -1e9
        qsl, ksl = [], []
        for hh in range(2):
            qs_h = np.concatenate(
                [q[:, heads[hh], :].T.astype(np.float32), wrow], axis=0)
            ks_h = np.concatenate(
                [ksb[:, hh * 64:(hh + 1) * 64].T, ind], axis=0)
            qsl.append(np.ascontiguousarray(qs_h).astype(bf))
            ksl.append(np.ascontiguousarray(ks_h).astype(bf))
        # vaug [128, 32*130]: pair a, token p -> [V_h0 | 1 | V_h1 | 1]
        vv = v[:, heads, :].reshape(32, 128, 128)   # [a, p, hd]
        vaug = np.ones((128, 32, 130), np.float32)
        vaug[:, :, 0:64] = vv.transpose(1, 0, 2)[:, :, 0:64]
        vaug[:, :, 65:129] = vv.transpose(1, 0, 2)[:, :, 64:128]
        # vsaug [128, 4*130]: pair pr: partitions 0-63 = block 2pr, 64-127 =
        # block 2pr+1
        vsp = vsb.reshape(4, 2, BS, 128).transpose(1, 2, 0, 3).reshape(128, 4, 128)
        vsaug = np.ones((128, 4, 130), np.float32)
        vsaug[:, :, 0:64] = vsp[:, :, 0:64]
        vsaug[:, :, 65:129] = vsp[:, :, 64:128]
        # vtail [64, 32*130]: odd blocks 2a+1
        vt = v[:, heads, :].reshape(32, 2, 64, 128)[:, 1]   # [a, p, hd]
        vtail = np.ones((64, 32, 130), np.float32)
        vtail[:, :, 0:64] = vt.transpose(1, 0, 2)[:, :, 0:64]
        vtail[:, :, 65:129] = vt.transpose(1, 0, 2)[:, :, 64:128]
        maps.append({"qT": qT, "kT": kT,
                     "qs0": qsl[0], "qs1": qsl[1],
                     "ks0": ksl[0], "ks1": ksl[1],
                     "vaug": vaug.reshape(128, 32 * 130).astype(bf),
                     "vsaug": vsaug.reshape(128, 4 * 130).astype(bf),
                     "vtail": vtail.reshape(64, 32 * 130).astype(bf)})
    return maps


def kernel(q, k, v, cu_seqlens_k=None, **_):
    from concourse.bass_utils import run_bass_kernel_spmd

    q = np.asarray(q, np.float32)
    k = np.asarray(k, np.float32)
    v = np.asarray(v, np.float32)
    if "nc" not in _cache:
        _cache["nc"] = _legalize_waits(_build_program())
    res = run_bass_kernel_spmd(_cache["nc"], _in_maps(q, k, v),
                               list(range(NCORES))).results
    out = np.empty((SEQ, N_HEADS, HEAD), np.float32)
    for c in range(NCORES):
        o = res[c]["outT"]                      # [130, SEQ]
        for hh, head in ((0, c), (1, c + 8)):
            num = o[hh * 65:hh * 65 + 64, :]    # [64, SEQ]
            den = o[hh * 65 + 64, :]            # [SEQ]
            out[:, head, :] = (num / den).T
    return out
